# revision 1
# baseline (speedup 1.0000x reference)
"""Bass/Tile TRN2 kernel for nn_BinaryTreeLSTM (B=256, L=32, D=512, H=512).

Incremental greedy TreeLSTM, data-parallel over batch (32 seqs/core, 8 cores).

Instead of recomputing all adjacent-pair compositions each of the 31 shrink
iterations (O(L^2) matmul work), maintains per-sequence state in "slot space"
(no physical shifting) plus an entity table of h|c value rows in DRAM:
  rows b*128+e: e in [0,32) leaf states, [32,63) initial pair candidates,
  [63,125) fresh candidates (2 per iteration).
Each iteration: argmax over maintained logits -> linked-list bookkeeping row
ops -> one indirect-DMA gather of 3 entity rows per sequence -> PE transposes
to feature-major -> compose matmul for just the 2 fresh pairs -> gating ->
fresh logits + masked scatter updates -> table write. All fp32 (greedy argmax
selection is precision-sensitive; bf16/tf32 matmuls flip merge decisions).
"""

import math
import sys
import numpy as np

sys.path.insert(0, "/opt/trn_rl_repo")

from contextlib import ExitStack

import concourse.bass as bass
import concourse.tile as tile
from concourse import bacc, mybir
from concourse.bass_utils import run_bass_kernel_spmd
from concourse.masks import make_identity
from concourse.tile import add_dep_helper

FP32 = mybir.dt.float32
U32 = mybir.dt.uint32
U8 = mybir.dt.uint8

B, L, D, H = 256, 32, 512, 512
NCORES = 8
BC = B // NCORES          # 32 sequences per core
NE = 128                  # entity rows per sequence in the table
NEG = -1.0e9
SENT = 33.0
NP3 = 96                  # 3 replicated bookkeeping groups of 32 partitions
AF = mybir.ActivationFunctionType
OP = mybir.AluOpType
AX = mybir.AxisListType
NIC = L - 1               # 31 initial candidates / iterations
DEBUG = False


def build_kernel(ctx: ExitStack, tc: "tile.TileContext", io: dict, sched):
    nc = tc.nc
    imax = max(i for i in range(NIC) if sched[i] >= 1) + 1

    # rotate bulk DMAs across engine queues; gathers get explicit deps
    dmaq = [nc.sync, nc.scalar]
    qi = [0]

    def qdma(**kw):
        eng = dmaq[qi[0] % 2]
        qi[0] += 1
        return eng.dma_start(**kw)

    xT = io["xT"]          # [4, 128, BC*L]
    wwT = io["wwT"]        # [4, 128, 2H]
    wcT = io["wcT"]        # [8, 128, 5H]
    bw_d = io["bw"]        # [128, 8]
    bc_d = io["bc"]        # [128, 20]  (+1.0 folded into fl/fr)
    qrep_d = io["qrep"]    # [128, 4, 96] scaled comp_query replicated over M
    tab = io["tab"]        # [BC*NE, 2H] value table (zero-filled)
    iota_d = io["iota32"]  # [96, 32]
    bcol_d = io["bcol"]    # [96, 1]  (p%32)*NE
    md2_d = io["mdiag2"]   # [96, 64]
    md31_d = io["mdiag31"] # [96, 992]
    lmask_d = io["lmask"]  # [96, 32]
    nxt0_d = io["nxt0"]    # [96, 32]
    prv0_d = io["prv0"]    # [96, 32]
    done_d = io["done"]    # [96, 31]
    out_d = io["out"]      # [BC, H]

    tabv_be = tab.rearrange("(b e) d -> b e d", b=BC)   # [32, 128, 1024]
    tabv_eb = tab.rearrange("(b e) d -> e b d", b=BC)   # [128, 32, 1024]

    consts = ctx.enter_context(tc.tile_pool(name="consts", bufs=1))
    state = ctx.enter_context(tc.tile_pool(name="state", bufs=1))

    # ---- persistent constants ----
    wc_sb = consts.tile([128, 8, 5 * H], FP32, tag="wc")
    for kt in range(8):
        nc.sync.dma_start(out=wc_sb[:, kt, :], in_=wcT[kt])
    bw_sb = consts.tile([128, 8], FP32, tag="bw")
    nc.sync.dma_start(out=bw_sb[:], in_=bw_d[:])
    bc_sb = consts.tile([128, 20], FP32, tag="bc")
    nc.sync.dma_start(out=bc_sb[:], in_=bc_d[:])
    qrep_sb = consts.tile([128, 4, 96], FP32, tag="qrep")
    nc.sync.dma_start(out=qrep_sb[:], in_=qrep_d[:])
    iota_sb = consts.tile([NP3, 32], FP32, tag="iota")
    nc.sync.dma_start(out=iota_sb[:], in_=iota_d[:])
    bcol_sb = consts.tile([NP3, 1], FP32, tag="bcol")
    nc.sync.dma_start(out=bcol_sb[:], in_=bcol_d[:])
    md2_sb = consts.tile([NP3, 64], FP32, tag="md2")
    nc.sync.dma_start(out=md2_sb[:], in_=md2_d[:])
    lmask_sb = consts.tile([NP3, 32], FP32, tag="lmask")
    nc.sync.dma_start(out=lmask_sb[:], in_=lmask_d[:])
    done_sb = consts.tile([NP3, NIC], FP32, tag="done")
    nc.sync.dma_start(out=done_sb[:], in_=done_d[:])
    negc_sb = consts.tile([NP3, 1], FP32, tag="negc")
    nc.vector.memset(negc_sb[:], NEG)
    bcrow_sb = consts.tile([1, 5 * H], FP32, tag="bcrow")
    nc.sync.dma_start(out=bcrow_sb[:], in_=io["bcrow"])
    ones_sb = consts.tile([1, 64], FP32, tag="ones")
    nc.vector.memset(ones_sb[:], 1.0)
    ident = consts.tile([128, 128], FP32, tag="ident")
    make_identity(nc, ident[:])

    # ---- bookkeeping state (3 replicated groups of 32 partitions) ----
    l_t = state.tile([NP3, 32], FP32, tag="l")
    nxt_t = state.tile([NP3, 32], FP32, tag="nxt")
    nc.sync.dma_start(out=nxt_t[:], in_=nxt0_d[:])
    prv_t = state.tile([NP3, 32], FP32, tag="prv")
    nc.sync.dma_start(out=prv_t[:], in_=prv0_d[:])
    loc_t = state.tile([NP3, 32], FP32, tag="loc")
    nc.vector.tensor_copy(loc_t[:], iota_sb[:])
    cidx_t = state.tile([NP3, 32], FP32, tag="cidx")
    nc.vector.tensor_scalar(cidx_t[:], iota_sb[:], 32.0, None, OP.add)
    nc.vector.memset(cidx_t[:, 31:32], 0.0)

    # ================= init phase =================
    with tc.tile_pool(name="initp", bufs=1) as initp, \
         tc.tile_pool(name="initx", bufs=2) as initx, \
         tc.tile_pool(name="initxb", bufs=1) as initxb, \
         tc.tile_pool(name="initps", bufs=5, space="PSUM") as initps, \
         tc.tile_pool(name="inittr", bufs=1, space="PSUM") as inittr:

        ww_sb = initp.tile([128, 4, 2 * H], FP32, tag="ww")
        for kt in range(4):
            nc.sync.dma_start(out=ww_sb[:, kt, :], in_=wwT[kt])
        md31_sb = initp.tile([NP3, 992], FP32, tag="md31")
        nc.sync.dma_start(out=md31_sb[:], in_=md31_d[:])

        # ---- leaves: hc = W_word @ x ; layout [128, f, b, s] ----
        hleaf = initp.tile([128, 4, BC, L], FP32, tag="hleaf")
        cleaf = initp.tile([128, 4, BC, L], FP32, tag="cleaf")
        for cidx2 in range(2):
            xb = []
            for kt in range(4):
                xt = initxb.tile([128, 512], FP32, tag=f"xb{kt}")
                nc.sync.dma_start(
                    out=xt[:], in_=xT[kt][:, cidx2 * 512:(cidx2 + 1) * 512])
                xb.append(xt)
            for m in range(8):
                ps = initps.tile([128, 512], FP32, tag="pg")
                for kt in range(4):
                    nc.tensor.matmul(
                        ps[:], ww_sb[:, kt, m * 128:(m + 1) * 128], xb[kt][:],
                        start=(kt == 0), stop=(kt == 3))
                dst = hleaf if m < 4 else cleaf
                dview = dst[:, m % 4, 16 * cidx2:16 * cidx2 + 16, :]
                nc.vector.tensor_scalar(
                    dview, ps[:].rearrange("p (b n) -> p b n", b=16),
                    bw_sb[:, m:m + 1], None, OP.add)

        # ---- write leaf rows to table (b-major) ----
        init_writes = []
        for blk in range(8):
            bm = initxb.tile([128, 8, 128], FP32, tag="bm")
            for m in range(8):
                src = hleaf if m < 4 else cleaf
                sv = src[:, m % 4].rearrange("p b s -> p (b s)")
                pt = inittr.tile([128, 128], FP32, tag="pt")
                nc.tensor.transpose(
                    pt[:], sv[:, blk * 128:(blk + 1) * 128], ident[:])
                nc.vector.tensor_copy(bm[:, m], pt[:])
            init_writes.append(qdma(
                out=tabv_be[4 * blk:4 * blk + 4, 0:32, :],
                in_=bm[:].rearrange("p m d -> p (m d)")))

        # ---- initial candidates: compose valid adjacent pairs, (s,b) order ----
        candh = initp.tile([128, 4, NIC * BC], FP32, tag="candh")
        candc = initp.tile([128, 4, NIC * BC], FP32, tag="candc")
        nc.vector.memset(candh[:], 0.0)
        nc.vector.memset(candc[:], 0.0)

        def leaf_v(t, kt, s0, s1, bh):
            return t[:, kt].rearrange("p b s -> p s b")[:, s0:s1, 0:bh]

        NTOT = NIC * BC  # 992
        # s-chunks with per-chunk active-b bound from the baked schedule
        chunks = []
        s0 = 0
        while s0 < imax:
            bh = max(sched[s0], 1)
            s1 = s0 + 1
            while s1 < imax and (s1 + 1 - s0) * bh <= 512:
                s1 += 1
            chunks.append((s0, s1, bh))
            s0 = s1
        for (sc0, sc1, bh) in chunks:
            ncol = (sc1 - sc0) * bh
            for f in range(4):
                ps_g = []
                for g in range(5):
                    mt = g * 4 + f
                    ps = initps.tile([128, 512], FP32, tag="pg")
                    for kt in range(8):
                        if kt < 4:
                            rv = leaf_v(hleaf, kt, sc0, sc1, bh)
                        else:
                            rv = leaf_v(hleaf, kt - 4, sc0 + 1, sc1 + 1, bh)
                        nc.tensor.matmul(
                            ps[:, 0:ncol],
                            wc_sb[:, kt, mt * 128:(mt + 1) * 128], rv,
                            start=(kt == 0), stop=(kt == 7))
                    ps_g.append(ps)
                cl = leaf_v(cleaf, f, sc0, sc1, bh)
                cr = leaf_v(cleaf, f, sc0 + 1, sc1 + 1, bh)
                cn = candc[:, f].rearrange("p (s b) -> p s b", b=BC)[:, sc0:sc1, 0:bh]
                hn = candh[:, f].rearrange("p (s b) -> p s b", b=BC)[:, sc0:sc1, 0:bh]
                def v3(t):
                    return t[:, 0:ncol].rearrange("p (s b) -> p s b", b=bh)

                si = initx.tile([128, 512], FP32, tag="si")
                nc.scalar.activation(si[:, 0:ncol], ps_g[0][:, 0:ncol],
                                     AF.Sigmoid, bias=bc_sb[:, f:f + 1])
                t1 = initx.tile([128, 512], FP32, tag="tg")
                nc.scalar.activation(t1[:, 0:ncol], ps_g[1][:, 0:ncol],
                                     AF.Sigmoid, bias=bc_sb[:, 4 + f:5 + f])
                nc.vector.tensor_tensor(cn, v3(t1), cl, OP.mult)
                t2 = initx.tile([128, 512], FP32, tag="tg")
                nc.scalar.activation(t2[:, 0:ncol], ps_g[2][:, 0:ncol],
                                     AF.Sigmoid, bias=bc_sb[:, 8 + f:9 + f])
                nc.gpsimd.tensor_tensor(v3(t2), v3(t2), cr, OP.mult)
                nc.gpsimd.tensor_tensor(cn, cn, v3(t2), OP.add)
                t3 = initx.tile([128, 512], FP32, tag="tg")
                nc.scalar.activation(t3[:, 0:ncol], ps_g[3][:, 0:ncol],
                                     AF.Tanh, bias=bc_sb[:, 12 + f:13 + f])
                nc.gpsimd.tensor_tensor(v3(t3), v3(t3), v3(si), OP.mult)
                nc.gpsimd.tensor_tensor(cn, cn, v3(t3), OP.add)
                so = initx.tile([128, 512], FP32, tag="so")
                nc.scalar.activation(so[:, 0:ncol], ps_g[4][:, 0:ncol],
                                     AF.Sigmoid, bias=bc_sb[:, 16 + f:17 + f])
                tcn = initx.tile([128, 512], FP32, tag="tg")
                nc.scalar.activation(v3(tcn), cn, AF.Tanh)
                nc.vector.tensor_tensor(hn, v3(so), v3(tcn), OP.mult)

        # ---- init logits ----
        nc.vector.memset(l_t[:], NEG)
        md31v = md31_sb[:].rearrange("p (s b) -> p s b", b=BC)
        for (sc0, sc1, bh) in chunks:
            ncol = (sc1 - sc0) * bh
            pl = inittr.tile([NP3, 512], FP32, tag="pl")
            chv = candh[:].rearrange("p f (s b) -> p f s b", b=BC)
            for f in range(4):
                nc.tensor.matmul(
                    pl[:, 0:ncol], qrep_sb[:, f, :],
                    chv[:, f, sc0:sc1, 0:bh],
                    start=(f == 0), stop=(f == 3))
            lm = initxb.tile([NP3, 512], FP32, tag="lm")
            nc.vector.tensor_tensor(
                lm[:, 0:ncol].rearrange("p (s b) -> p s b", b=bh),
                pl[:, 0:ncol].rearrange("p (s b) -> p s b", b=bh),
                md31v[:, sc0:sc1, 0:bh], OP.mult)
            nc.vector.tensor_reduce(
                l_t[:, sc0:sc1],
                lm[:, 0:ncol].rearrange("p (s b) -> p s b", b=bh),
                AX.X, OP.add)
        # mask invalid: l = l*lmask + (1-lmask)*NEG; (lmask-1)*(-NEG) is that term
        tmpl = initx.tile([NP3, 32], FP32, tag="tmpl")
        nc.vector.tensor_scalar(tmpl[:], lmask_sb[:], 1.0, -NEG, OP.subtract,
                                OP.mult)
        nc.vector.tensor_tensor(l_t[:], l_t[:], lmask_sb[:], OP.mult)
        nc.vector.tensor_tensor(l_t[:], l_t[:], tmpl[:], OP.add)

        # ---- write init candidate rows to table ((s,b) order) ----
        for blk in range(8):
            c0 = blk * 128
            c1 = min(c0 + 128, NTOT)
            w = c1 - c0
            bm = initxb.tile([128, 8, 128], FP32, tag="bm")
            for m in range(8):
                src = candh if m < 4 else candc
                pt = inittr.tile([128, 128], FP32, tag="pt")
                nc.tensor.transpose(
                    pt[0:w, :], src[:, m % 4, c0:c1], ident[:])
                nc.vector.tensor_copy(bm[0:w, m], pt[0:w, :])
            s0 = 32 + 4 * blk
            s1 = 32 + min(4 * blk + 4, NIC)
            init_writes.append(qdma(
                out=tabv_eb[s0:s1, 0:32, :],
                in_=bm[0:w].rearrange("p m d -> p (m d)")))

    # ================= iterations =================
    itp = ctx.enter_context(tc.tile_pool(name="itp", bufs=2))
    itk = ctx.enter_context(tc.tile_pool(name="itk", bufs=2))
    gps = ctx.enter_context(tc.tile_pool(name="gps", bufs=3, space="PSUM"))
    trs = ctx.enter_context(tc.tile_pool(name="trs", bufs=2, space="PSUM"))
    ops = ctx.enter_context(tc.tile_pool(name="ops", bufs=1, space="PSUM"))

    dbg = io.get("dbg")
    G = state.tile([NP3, 2 * H], FP32, tag="G")
    last_writes = init_writes[-3:]
    for i in range(imax):
        ab = min(int(sched[i]), 32)   # active sequences this iteration
        dcol = done_sb[:, i:i + 1]
        if dbg is not None:
            nc.sync.dma_start(out=dbg["l"][i], in_=l_t[:])

        # -- 1. argmax over logits --
        mx8 = itk.tile([NP3, 8], FP32, tag="mx8")
        nc.vector.max(mx8[:], l_t[:])
        ix8 = itk.tile([NP3, 8], U32, tag="ix8")
        nc.vector.max_index(ix8[:], mx8[:], l_t[:])
        sstar = itk.tile([NP3, 1], FP32, tag="sstar")
        nc.vector.tensor_copy(sstar[:], ix8[:, 0:1])

        # -- 2. linked-list row gathers --
        ohs = itk.tile([NP3, 32], FP32, tag="ohs")
        nc.vector.tensor_scalar(ohs[:], iota_sb[:], sstar[:], None,
                                OP.is_equal)
        tmp = itk.tile([NP3, 32], FP32, tag="tmp")

        def rowgather(arr, oh, name):
            col = itk.tile([NP3, 1], FP32, tag=name)
            nc.vector.tensor_tensor(tmp[:], oh, arr, OP.mult)
            nc.vector.tensor_reduce(col[:], tmp[:], AX.X, OP.max)
            return col

        nstar = rowgather(nxt_t[:], ohs[:], "nstar")
        pstar = rowgather(prv_t[:], ohs[:], "pstar")
        ci = rowgather(cidx_t[:], ohs[:], "ci")
        ohn = itk.tile([NP3, 32], FP32, tag="ohn")
        nc.vector.tensor_scalar(ohn[:], iota_sb[:], nstar[:], None,
                                OP.is_equal)
        n2 = rowgather(nxt_t[:], ohn[:], "n2")
        ohp = itk.tile([NP3, 32], FP32, tag="ohp")
        nc.vector.tensor_scalar(ohp[:], iota_sb[:], pstar[:], None,
                                OP.is_equal)
        e1c = rowgather(loc_t[:], ohp[:], "e1c")
        ohn2 = itk.tile([NP3, 32], FP32, tag="ohn2")
        nc.vector.tensor_scalar(ohn2[:], iota_sb[:], n2[:], None, OP.is_equal)
        e3c = rowgather(loc_t[:], ohn2[:], "e3c")

        # -- 3. gather index column [96,1]: groups (E1 | E2 | E3) + b*NE --
        gsel = itk.tile([NP3, 1], FP32, tag="gsel")
        nc.vector.tensor_copy(gsel[0:32, :], e1c[0:32, :])
        nc.vector.tensor_copy(gsel[32:64, :], ci[32:64, :])
        nc.vector.tensor_copy(gsel[64:96, :], e3c[64:96, :])
        nc.vector.tensor_tensor(gsel[:], gsel[:], bcol_sb[:], OP.add)
        gidx = itk.tile([NP3, 1], U32, tag="gidx")
        nc.vector.tensor_copy(gidx[:], gsel[:])
        if dbg is not None:
            nc.sync.dma_start(out=dbg["ss"][i], in_=sstar[:])
            nc.sync.dma_start(out=dbg["gs"][i], in_=gsel[:])

        # -- 4. indirect gather of entity rows (must order after table writes;
        #       Tile does not track RAW hazards through DRAM) --
        if False:
            for p0 in (0, 32, 64):
                gins = nc.gpsimd.indirect_dma_start(
                    out=G[p0:p0 + ab, :], out_offset=None, in_=tab,
                    in_offset=bass.IndirectOffsetOnAxis(
                        ap=gidx[p0:p0 + ab, :1], axis=0))
                for wr in last_writes:
                    add_dep_helper(gins.ins, wr.ins, reason="table RAW")
        else:
            gins = nc.gpsimd.indirect_dma_start(
                out=G[:], out_offset=None, in_=tab,
                in_offset=bass.IndirectOffsetOnAxis(ap=gidx[:, :1], axis=0))
            for wr in last_writes:
                add_dep_helper(gins.ins, wr.ins, reason="table RAW")

        # -- masks + pointer updates (independent of the compose results;
        #    run them here so they overlap the gather/matmul) --
        ohs_d = itk.tile([NP3, 32], FP32, tag="ohsd")
        nc.vector.tensor_scalar(ohs_d[:], iota_sb[:], sstar[:], dcol,
                                OP.is_equal, OP.mult)
        ohs_d8 = itk.tile([NP3, 32], U8, tag="ohsd8")
        nc.vector.tensor_copy(ohs_d8[:], ohs_d[:])
        m1 = itk.tile([NP3, 32], FP32, tag="m1")
        nc.vector.tensor_scalar(m1[:], iota_sb[:], pstar[:], dcol,
                                OP.is_equal, OP.mult)
        m1_8 = itk.tile([NP3, 32], U8, tag="m18")
        nc.vector.tensor_copy(m1_8[:], m1[:])
        ohn_d8 = itk.tile([NP3, 32], U8, tag="ohnd8")
        nc.vector.tensor_scalar(tmp[:], iota_sb[:], nstar[:], dcol,
                                OP.is_equal, OP.mult)
        nc.vector.tensor_copy(ohn_d8[:], tmp[:])
        ohn2_d8 = itk.tile([NP3, 32], U8, tag="ohn2d8")
        nc.vector.tensor_scalar(tmp[:], iota_sb[:], n2[:], dcol,
                                OP.is_equal, OP.mult)
        nc.vector.tensor_copy(ohn2_d8[:], tmp[:])
        ex2 = itk.tile([NP3, 1], FP32, tag="ex2")
        nc.vector.tensor_scalar(ex2[:], n2[:], SENT, None, OP.is_lt)

        # loc[s*] = ci ; nxt[s*] = n2 ; prv[n2] = s* ; cidx[p*/s*] = e1/e2
        e1v, e2v = float(63 + 2 * i), float(64 + 2 * i)
        nc.vector.copy_predicated(loc_t[:], ohs_d8[:],
                                  ci[:].to_broadcast([NP3, 32]))
        nc.vector.copy_predicated(nxt_t[:], ohs_d8[:],
                                  n2[:].to_broadcast([NP3, 32]))
        nc.vector.copy_predicated(prv_t[:], ohn2_d8[:],
                                  sstar[:].to_broadcast([NP3, 32]))
        cst = itk.tile([NP3, 1], FP32, tag="cst")
        nc.vector.memset(cst[:], e1v)
        nc.vector.copy_predicated(cidx_t[:], m1_8[:],
                                  cst[:].to_broadcast([NP3, 32]))
        cst2 = itk.tile([NP3, 1], FP32, tag="cst2")
        nc.vector.memset(cst2[:], e2v)
        nc.vector.copy_predicated(cidx_t[:], ohs_d8[:],
                                  cst2[:].to_broadcast([NP3, 32]))

        # -- 5. transpose to feature-major --
        Th = itp.tile([128, 4, NP3], FP32, tag="Th")
        Tc = itp.tile([128, 4, NP3], FP32, tag="Tc")
        for f in range(4):
            pt = trs.tile([128, 128], FP32, tag="pt")
            nc.tensor.transpose(pt[:, 0:NP3], G[:, f * 128:(f + 1) * 128],
                                ident[0:NP3, 0:NP3])
            nc.vector.tensor_copy(Th[:, f], pt[:, 0:NP3])
            pt2 = trs.tile([128, 128], FP32, tag="pt")
            nc.tensor.transpose(pt2[:, 0:NP3],
                                G[:, 512 + f * 128:512 + (f + 1) * 128],
                                ident[0:NP3, 0:NP3])
            nc.scalar.copy(Tc[:, f], pt2[:, 0:NP3])

        # -- 6. compose matmul for the 2 fresh pairs (cols: pair1 b | pair2 b) --
        # psum per gate holds all 4 f-tiles [128, (f,pair,b)=256]; bias folded
        # in via a K=1 matmul so one activation per gate covers all f.
        hcand = itp.tile([128, 4, 64], FP32, tag="hcand")
        ccand = itp.tile([128, 4, 64], FP32, tag="ccand")
        # active sequences are a sorted prefix: pair1 occupies cols [0:32],
        # pair2's active lanes [32:32+ab] -> one contiguous N=32+ab slice.
        nmm = 32 + ab
        acts = []
        for g in range(5):
            ps = gps.tile([128, 512], FP32, tag="pg")
            for f in range(4):
                mt = g * 4 + f
                nc.tensor.matmul(
                    ps[:, f * 64:f * 64 + 64],
                    bcrow_sb[0:1, mt * 128:(mt + 1) * 128],
                    ones_sb[0:1, 0:64], start=True, stop=False)
                for kt in range(8):
                    rv = (Th[:, kt, 0:nmm] if kt < 4
                          else Th[:, kt - 4, 32:32 + nmm])
                    nc.tensor.matmul(
                        ps[:, f * 64:f * 64 + nmm],
                        wc_sb[:, kt, mt * 128:(mt + 1) * 128],
                        rv, start=False, stop=(kt == 7))
            a = itk.tile([128, 256], FP32, tag=f"ga{g}")
            nc.scalar.activation(a[:], ps[:, 0:256],
                                 AF.Tanh if g == 3 else AF.Sigmoid)
            acts.append(a)
        si, t1, t2, t3, so = acts

        def a3(t):
            return t[:].rearrange("p (f n) -> p f n", n=64)

        cl = Tc[:, :, 0:64]
        cr = Tc[:, :, 32:96]
        cn = ccand[:]
        hn = hcand[:]
        nc.vector.tensor_tensor(cn, a3(t1), cl, OP.mult)
        nc.gpsimd.tensor_tensor(a3(t2), a3(t2), cr, OP.mult)
        nc.gpsimd.tensor_tensor(cn, cn, a3(t2), OP.add)
        nc.gpsimd.tensor_tensor(a3(t3), a3(t3), a3(si), OP.mult)
        nc.vector.tensor_tensor(cn, cn, a3(t3), OP.add)
        tcn = itk.tile([128, 256], FP32, tag="tg")
        nc.scalar.activation(a3(tcn), cn, AF.Tanh)
        nc.vector.tensor_tensor(hn, a3(so), a3(tcn), OP.mult)

        # -- 7. fresh logits --
        pl = ops.tile([NP3, 64], FP32, tag="pl")
        for f in range(4):
            nc.tensor.matmul(pl[:], qrep_sb[:, f, :], hcand[:, f, :],
                             start=(f == 0), stop=(f == 3))
        lmt = itk.tile([NP3, 64], FP32, tag="lmt")
        nc.vector.tensor_tensor(lmt[:], pl[:], md2_sb[:], OP.mult)
        lnew = itk.tile([NP3, 2], FP32, tag="lnew")
        nc.vector.tensor_reduce(
            lnew[:], lmt[:].rearrange("p (r b) -> p r b", r=2), AX.X, OP.add)
        if dbg is not None:
            nc.sync.dma_start(out=dbg["ln"][i], in_=lnew[:])

        # -- 8. write fresh candidate rows to table --
        W = itp.tile([64, 8, 128], FP32, tag="W")
        for f in range(4):
            po = trs.tile([128, 128], FP32, tag="pt")
            nc.tensor.transpose(po[0:64, :], hcand[:, f, :], ident[:])
            nc.vector.tensor_copy(W[:, f], po[0:64, :])
            po2 = trs.tile([128, 128], FP32, tag="pt")
            nc.tensor.transpose(po2[0:64, :], ccand[:, f, :], ident[:])
            nc.vector.tensor_copy(W[:, 4 + f], po2[0:64, :])
        w1 = qdma(out=tabv_eb[63 + 2 * i, 0:ab, :],
                  in_=W[0:ab].rearrange("p m d -> p (m d)"))
        w2 = qdma(out=tabv_eb[64 + 2 * i, 0:ab, :],
                  in_=W[32:32 + ab].rearrange("p m d -> p (m d)"))
        last_writes = [w1, w2]

        # -- 9. logit updates (only remaining post-compose bookkeeping) --
        # l[p*] = v1 ; l[s*] = v2 if n2 exists else NEG ; l[n*] = NEG
        nc.vector.copy_predicated(l_t[:], m1_8[:],
                                  lnew[:, 0:1].to_broadcast([NP3, 32]))
        # v2p = v2*ex2 + NEG*(1-ex2), avoiding 1e9 absorption of v2
        v2p = itk.tile([NP3, 1], FP32, tag="v2p")
        nc.vector.tensor_tensor(v2p[:], lnew[:, 1:2], ex2[:], OP.mult)
        negpart = itk.tile([NP3, 1], FP32, tag="negpart")
        nc.vector.tensor_scalar(negpart[:], ex2[:], 1.0, -NEG, OP.subtract,
                                OP.mult)
        nc.vector.tensor_tensor(v2p[:], v2p[:], negpart[:], OP.add)
        nc.vector.copy_predicated(l_t[:], ohs_d8[:],
                                  v2p[:].to_broadcast([NP3, 32]))
        nc.vector.copy_predicated(l_t[:], ohn_d8[:],
                                  negc_sb[:].to_broadcast([NP3, 32]))

    # ================= output =================
    oidx = itk.tile([BC, 1], FP32, tag="oidx")
    nc.vector.tensor_tensor(oidx[:], loc_t[0:BC, 0:1], bcol_sb[0:BC, :],
                            OP.add)
    oidxu = itk.tile([BC, 1], U32, tag="oidxu")
    nc.vector.tensor_copy(oidxu[:], oidx[:])
    Gout = itp.tile([BC, 2 * H], FP32, tag="Gout")
    gout_ins = nc.gpsimd.indirect_dma_start(
        out=Gout[:], out_offset=None, in_=tab,
        in_offset=bass.IndirectOffsetOnAxis(ap=oidxu[:, :1], axis=0))
    for wr in last_writes:
        add_dep_helper(gout_ins.ins, wr.ins, reason="table RAW")
    nc.sync.dma_start(out=out_d, in_=Gout[:, 0:H])


_BUILD_CACHE = {}


def build(sched=None):
    if sched is None:
        sched = (32,) * NIC
    sched = tuple(int(v) for v in sched)
    if sched in _BUILD_CACHE:
        return _BUILD_CACHE[sched]
    nc = bacc.Bacc("TRN2", target_bir_lowering=False, debug=False)
    io = {
        "xT": nc.dram_tensor("xT", [4, 128, BC * L], FP32, kind="ExternalInput").ap(),
        "wwT": nc.dram_tensor("wwT", [4, 128, 2 * H], FP32, kind="ExternalInput").ap(),
        "wcT": nc.dram_tensor("wcT", [8, 128, 5 * H], FP32, kind="ExternalInput").ap(),
        "bw": nc.dram_tensor("bw", [128, 8], FP32, kind="ExternalInput").ap(),
        "bc": nc.dram_tensor("bc", [128, 20], FP32, kind="ExternalInput").ap(),
        "bcrow": nc.dram_tensor("bcrow", [1, 5 * H], FP32, kind="ExternalInput").ap(),
        "qrep": nc.dram_tensor("qrep", [128, 4, 96], FP32, kind="ExternalInput").ap(),
        "tab": nc.dram_tensor("tab", [BC * NE, 2 * H], FP32, kind="ExternalInput").ap(),
        "iota32": nc.dram_tensor("iota32", [NP3, 32], FP32, kind="ExternalInput").ap(),
        "bcol": nc.dram_tensor("bcol", [NP3, 1], FP32, kind="ExternalInput").ap(),
        "mdiag2": nc.dram_tensor("mdiag2", [NP3, 64], FP32, kind="ExternalInput").ap(),
        "mdiag31": nc.dram_tensor("mdiag31", [NP3, NIC * BC], FP32, kind="ExternalInput").ap(),
        "lmask": nc.dram_tensor("lmask", [NP3, 32], FP32, kind="ExternalInput").ap(),
        "nxt0": nc.dram_tensor("nxt0", [NP3, 32], FP32, kind="ExternalInput").ap(),
        "prv0": nc.dram_tensor("prv0", [NP3, 32], FP32, kind="ExternalInput").ap(),
        "done": nc.dram_tensor("done", [NP3, NIC], FP32, kind="ExternalInput").ap(),
        "out": nc.dram_tensor("out", [BC, H], FP32, kind="ExternalOutput").ap(),
    }
    if DEBUG:
        io["dbg"] = {
            "l": nc.dram_tensor("dbg_l", [NIC, NP3, 32], FP32, kind="ExternalOutput").ap(),
            "ss": nc.dram_tensor("dbg_ss", [NIC, NP3, 1], FP32, kind="ExternalOutput").ap(),
            "gs": nc.dram_tensor("dbg_gs", [NIC, NP3, 1], FP32, kind="ExternalOutput").ap(),
            "ln": nc.dram_tensor("dbg_ln", [NIC, NP3, 2], FP32, kind="ExternalOutput").ap(),
        }
    with tile.TileContext(nc) as tc:
        with ExitStack() as ctx:
            build_kernel(ctx, tc, io, sched)
    nc.compile()
    _BUILD_CACHE[sched] = nc
    return nc


def make_sched(length):
    length = np.asarray(length).astype(np.int64)
    cnt = [(length > i + 1).sum() for i in range(NIC)]
    return tuple(int(-(-c // NCORES)) for c in cnt)


def make_order(length):
    length = np.asarray(length).astype(np.int64)
    order = np.argsort(-length, kind="stable")
    return order.reshape(L, NCORES)


def make_in_maps(x, length, W_word, b_word, W_comp, b_comp, comp_query):
    x = np.asarray(x, np.float32)
    length = np.asarray(length).astype(np.int64)
    W_word = np.asarray(W_word, np.float32)
    b_word = np.asarray(b_word, np.float32)
    W_comp = np.asarray(W_comp, np.float32)
    b_comp = np.asarray(b_comp, np.float32)
    comp_query = np.asarray(comp_query, np.float32)

    wwT = np.ascontiguousarray(W_word.T.reshape(4, 128, 2 * H))
    wcT = np.ascontiguousarray(W_comp.T.reshape(8, 128, 5 * H))
    bw = np.ascontiguousarray(b_word.reshape(8, 128).T)
    bca = b_comp.copy()
    bca[H:3 * H] += 1.0
    bc = np.ascontiguousarray(bca.reshape(20, 128).T)
    qs = (comp_query * (1.0 / np.sqrt(H))).astype(np.float32)
    qrep = np.ascontiguousarray(
        np.broadcast_to(qs.reshape(4, 128, 1), (4, 128, NP3))
        .transpose(1, 0, 2)).astype(np.float32)

    iota32 = np.tile(np.arange(32, dtype=np.float32), (NP3, 1))
    bcol = (np.arange(NP3, dtype=np.float32) % BC).reshape(NP3, 1) * NE
    bidx = np.arange(NP3) % BC
    md2 = np.zeros((NP3, 64), np.float32)
    md2[np.arange(NP3), bidx] = 1.0
    md2[np.arange(NP3), 32 + bidx] = 1.0
    md31 = np.zeros((NP3, NIC, BC), np.float32)
    md31[np.arange(NP3), :, bidx] = 1.0
    md31 = md31.reshape(NP3, NIC * BC)
    tabz = np.zeros((BC * NE, 2 * H), np.float32)

    ordmat = make_order(length)
    in_maps = []
    for k in range(NCORES):
        idxs = ordmat[:, k]
        xs = x[idxs]
        xT = np.ascontiguousarray(xs.transpose(2, 0, 1).reshape(4, 128, BC * L))
        ln = length[idxs].astype(np.int64)
        lnr = ln[bidx]  # [96]
        lmask = (np.arange(32)[None, :] < (lnr[:, None] - 1)).astype(np.float32)
        lmask[:, 31] = 0.0
        nxt0 = np.full((NP3, 32), SENT, np.float32)
        prv0 = np.full((NP3, 32), SENT, np.float32)
        for p in range(NP3):
            m = int(lnr[p])
            for s in range(m - 1):
                nxt0[p, s] = s + 1
            for s in range(1, m):
                prv0[p, s] = s - 1
        done = (np.arange(1, L)[None, :] < lnr[:, None]).astype(np.float32)
        in_maps.append({
            "xT": xT, "wwT": wwT, "wcT": wcT, "bw": bw, "bc": bc,
            "bcrow": np.ascontiguousarray(bca.reshape(1, 5 * H)),
            "qrep": qrep, "tab": tabz, "iota32": iota32, "bcol": bcol,
            "mdiag2": md2, "mdiag31": md31, "lmask": lmask,
            "nxt0": nxt0, "prv0": prv0, "done": done,
        })
    return in_maps


def kernel(x, length, W_word, b_word, W_comp, b_comp, comp_query):
    nc = build(make_sched(length))
    in_maps = make_in_maps(x, length, W_word, b_word, W_comp, b_comp, comp_query)
    res = run_bass_kernel_spmd(nc, in_maps, list(range(NCORES)))
    out = np.zeros((B, H), np.float32)
    ordmat = make_order(length)
    for k in range(NCORES):
        out[ordmat[:, k]] = res.results[k]["out"]
    return out



# revision 3
# speedup vs baseline: 1.0593x; 1.0593x over previous
"""Bass/Tile TRN2 kernel for nn_BinaryTreeLSTM (B=256, L=32, D=512, H=512).

Incremental greedy TreeLSTM, data-parallel over batch (32 seqs/core, 8 cores).

Instead of recomputing all adjacent-pair compositions each of the 31 shrink
iterations (O(L^2) matmul work), maintains per-sequence state in "slot space"
(no physical shifting) plus an entity table of h|c value rows in DRAM:
  rows b*128+e: e in [0,32) leaf states, [32,63) initial pair candidates,
  [63,125) fresh candidates (2 per iteration).
Each iteration: argmax over maintained logits -> linked-list bookkeeping row
ops -> one indirect-DMA gather of 3 entity rows per sequence -> PE transposes
to feature-major -> compose matmul for just the 2 fresh pairs -> gating ->
fresh logits + masked scatter updates -> table write. All fp32 (greedy argmax
selection is precision-sensitive; bf16/tf32 matmuls flip merge decisions).
"""

import math
import sys
import numpy as np

sys.path.insert(0, "/opt/trn_rl_repo")

from contextlib import ExitStack

import concourse.bass as bass
import concourse.tile as tile
from concourse import bacc, mybir
from concourse.bass_utils import run_bass_kernel_spmd
from concourse.masks import make_identity
from concourse.tile import add_dep_helper

FP32 = mybir.dt.float32
U32 = mybir.dt.uint32
U8 = mybir.dt.uint8

B, L, D, H = 256, 32, 512, 512
NCORES = 8
BC = B // NCORES          # 32 sequences per core
NE = 128                  # entity rows per sequence in the table
NEG = -1.0e9
SENT = 33.0
NP3 = 96                  # 3 replicated bookkeeping groups of 32 partitions
AF = mybir.ActivationFunctionType
OP = mybir.AluOpType
AX = mybir.AxisListType
NIC = L - 1               # 31 initial candidates / iterations
DEBUG = False


def build_kernel(ctx: ExitStack, tc: "tile.TileContext", io: dict, sched):
    nc = tc.nc
    imax = max(i for i in range(NIC) if sched[i] >= 1) + 1

    # rotate bulk DMAs across engine queues; gathers get explicit deps
    dmaq = [nc.sync, nc.scalar]
    qi = [0]

    def qdma(**kw):
        eng = dmaq[qi[0] % 2]
        qi[0] += 1
        return eng.dma_start(**kw)

    xT = io["xT"]          # [4, 128, BC*L]
    wwT = io["wwT"]        # [4, 128, 2H]
    wcT = io["wcT"]        # [8, 128, 5H]
    bw_d = io["bw"]        # [128, 8]
    bc_d = io["bc"]        # [128, 20]  (+1.0 folded into fl/fr)
    qrep_d = io["qrep"]    # [128, 4, 96] scaled comp_query replicated over M
    tab = io["tab"]        # [BC*NE, 2H] value table (zero-filled)
    iota_d = io["iota32"]  # [96, 32]
    bcol_d = io["bcol"]    # [96, 1]  (p%32)*NE
    md2_d = io["mdiag2"]   # [96, 64]
    md31_d = io["mdiag31"] # [96, 992]
    lmask_d = io["lmask"]  # [96, 32]
    nxt0_d = io["nxt0"]    # [96, 32]
    prv0_d = io["prv0"]    # [96, 32]
    done_d = io["done"]    # [96, 31]
    out_d = io["out"]      # [BC, H]

    tabv_be = tab.rearrange("(b e) d -> b e d", b=BC)   # [32, 128, 1024]
    tabv_eb = tab.rearrange("(b e) d -> e b d", b=BC)   # [128, 32, 1024]

    consts = ctx.enter_context(tc.tile_pool(name="consts", bufs=1))
    state = ctx.enter_context(tc.tile_pool(name="state", bufs=1))

    # ---- persistent constants ----
    wc_sb = consts.tile([128, 8, 5 * H], FP32, tag="wc")
    for kt in range(8):
        nc.sync.dma_start(out=wc_sb[:, kt, :], in_=wcT[kt])
    bw_sb = consts.tile([128, 8], FP32, tag="bw")
    nc.sync.dma_start(out=bw_sb[:], in_=bw_d[:])
    bc_sb = consts.tile([128, 20], FP32, tag="bc")
    nc.sync.dma_start(out=bc_sb[:], in_=bc_d[:])
    qrep_sb = consts.tile([128, 4, 96], FP32, tag="qrep")
    nc.sync.dma_start(out=qrep_sb[:], in_=qrep_d[:])
    iota_sb = consts.tile([NP3, 32], FP32, tag="iota")
    nc.sync.dma_start(out=iota_sb[:], in_=iota_d[:])
    bcol_sb = consts.tile([NP3, 1], FP32, tag="bcol")
    nc.sync.dma_start(out=bcol_sb[:], in_=bcol_d[:])
    md2_sb = consts.tile([NP3, 64], FP32, tag="md2")
    nc.sync.dma_start(out=md2_sb[:], in_=md2_d[:])
    lmask_sb = consts.tile([NP3, 32], FP32, tag="lmask")
    nc.sync.dma_start(out=lmask_sb[:], in_=lmask_d[:])
    done_sb = consts.tile([NP3, NIC], FP32, tag="done")
    nc.sync.dma_start(out=done_sb[:], in_=done_d[:])
    negc_sb = consts.tile([NP3, 1], FP32, tag="negc")
    nc.vector.memset(negc_sb[:], NEG)
    bcrow_sb = consts.tile([1, 5 * H], FP32, tag="bcrow")
    nc.sync.dma_start(out=bcrow_sb[:], in_=io["bcrow"])
    ones_sb = consts.tile([1, 64], FP32, tag="ones")
    nc.vector.memset(ones_sb[:], 1.0)
    ident = consts.tile([128, 128], FP32, tag="ident")
    make_identity(nc, ident[:])

    # ---- bookkeeping state (3 replicated groups of 32 partitions) ----
    l_t = state.tile([NP3, 32], FP32, tag="l")
    nxt_t = state.tile([NP3, 32], FP32, tag="nxt")
    nc.sync.dma_start(out=nxt_t[:], in_=nxt0_d[:])
    prv_t = state.tile([NP3, 32], FP32, tag="prv")
    nc.sync.dma_start(out=prv_t[:], in_=prv0_d[:])
    loc_t = state.tile([NP3, 32], FP32, tag="loc")
    nc.vector.tensor_copy(loc_t[:], iota_sb[:])
    cidx_t = state.tile([NP3, 32], FP32, tag="cidx")
    nc.vector.tensor_scalar(cidx_t[:], iota_sb[:], 32.0, None, OP.add)
    nc.vector.memset(cidx_t[:, 31:32], 0.0)

    # ================= init phase =================
    with tc.tile_pool(name="initp", bufs=1) as initp, \
         tc.tile_pool(name="initx", bufs=2) as initx, \
         tc.tile_pool(name="initxb", bufs=1) as initxb, \
         tc.tile_pool(name="initps", bufs=5, space="PSUM") as initps, \
         tc.tile_pool(name="inittr", bufs=1, space="PSUM") as inittr:

        ww_sb = initp.tile([128, 4, 2 * H], FP32, tag="ww")
        for kt in range(4):
            nc.sync.dma_start(out=ww_sb[:, kt, :], in_=wwT[kt])
        md31_sb = initp.tile([NP3, 992], FP32, tag="md31")
        nc.sync.dma_start(out=md31_sb[:], in_=md31_d[:])

        # ---- leaves: hc = W_word @ x ; layout [128, f, b, s] ----
        hleaf = initp.tile([128, 4, BC, L], FP32, tag="hleaf")
        cleaf = initp.tile([128, 4, BC, L], FP32, tag="cleaf")
        for cidx2 in range(2):
            xb = []
            for kt in range(4):
                xt = initxb.tile([128, 512], FP32, tag=f"xb{kt}")
                nc.sync.dma_start(
                    out=xt[:], in_=xT[kt][:, cidx2 * 512:(cidx2 + 1) * 512])
                xb.append(xt)
            for m in range(8):
                ps = initps.tile([128, 512], FP32, tag="pg")
                for kt in range(4):
                    nc.tensor.matmul(
                        ps[:], ww_sb[:, kt, m * 128:(m + 1) * 128], xb[kt][:],
                        start=(kt == 0), stop=(kt == 3))
                dst = hleaf if m < 4 else cleaf
                dview = dst[:, m % 4, 16 * cidx2:16 * cidx2 + 16, :]
                nc.vector.tensor_scalar(
                    dview, ps[:].rearrange("p (b n) -> p b n", b=16),
                    bw_sb[:, m:m + 1], None, OP.add)

        # ---- write leaf rows to table (b-major) ----
        # per-b 2D DMAs: 3D APs (and degenerate [1,1] dims from rearrange)
        # shatter descriptor merging and cost ~30x on the DMA engines.
        init_writes = []
        for blk in range(8):
            bm = initxb.tile([128, 1024], FP32, tag="bm")
            for m in range(8):
                src = hleaf if m < 4 else cleaf
                sv = src[:, m % 4].rearrange("p b s -> p (b s)")
                pt = inittr.tile([128, 128], FP32, tag="pt")
                nc.tensor.transpose(
                    pt[:], sv[:, blk * 128:(blk + 1) * 128], ident[:])
                nc.vector.tensor_copy(bm[:, m * 128:(m + 1) * 128], pt[:])
            for j in range(4):
                init_writes.append(qdma(
                    out=tabv_be[4 * blk + j, 0:32, :],
                    in_=bm[32 * j:32 * j + 32, :]))

        # ---- initial candidates: compose valid adjacent pairs, (s,b) order ----
        candh = initp.tile([128, 4, NIC * BC], FP32, tag="candh")
        candc = initp.tile([128, 4, NIC * BC], FP32, tag="candc")
        nc.vector.memset(candh[:], 0.0)
        nc.vector.memset(candc[:], 0.0)

        def leaf_v(t, kt, s0, s1, bh):
            return t[:, kt].rearrange("p b s -> p s b")[:, s0:s1, 0:bh]

        NTOT = NIC * BC  # 992
        # s-chunks with per-chunk active-b bound from the baked schedule
        chunks = []
        s0 = 0
        while s0 < imax:
            bh = max(sched[s0], 1)
            s1 = s0 + 1
            while s1 < imax and (s1 + 1 - s0) * bh <= 512:
                s1 += 1
            chunks.append((s0, s1, bh))
            s0 = s1
        for (sc0, sc1, bh) in chunks:
            ncol = (sc1 - sc0) * bh
            for f in range(4):
                ps_g = []
                for g in range(5):
                    mt = g * 4 + f
                    ps = initps.tile([128, 512], FP32, tag="pg")
                    for kt in range(8):
                        if kt < 4:
                            rv = leaf_v(hleaf, kt, sc0, sc1, bh)
                        else:
                            rv = leaf_v(hleaf, kt - 4, sc0 + 1, sc1 + 1, bh)
                        nc.tensor.matmul(
                            ps[:, 0:ncol],
                            wc_sb[:, kt, mt * 128:(mt + 1) * 128], rv,
                            start=(kt == 0), stop=(kt == 7))
                    ps_g.append(ps)
                cl = leaf_v(cleaf, f, sc0, sc1, bh)
                cr = leaf_v(cleaf, f, sc0 + 1, sc1 + 1, bh)
                cn = candc[:, f].rearrange("p (s b) -> p s b", b=BC)[:, sc0:sc1, 0:bh]
                hn = candh[:, f].rearrange("p (s b) -> p s b", b=BC)[:, sc0:sc1, 0:bh]
                def v3(t):
                    return t[:, 0:ncol].rearrange("p (s b) -> p s b", b=bh)

                si = initx.tile([128, 512], FP32, tag="si")
                nc.scalar.activation(si[:, 0:ncol], ps_g[0][:, 0:ncol],
                                     AF.Sigmoid, bias=bc_sb[:, f:f + 1])
                t1 = initx.tile([128, 512], FP32, tag="tg")
                nc.scalar.activation(t1[:, 0:ncol], ps_g[1][:, 0:ncol],
                                     AF.Sigmoid, bias=bc_sb[:, 4 + f:5 + f])
                nc.vector.tensor_tensor(cn, v3(t1), cl, OP.mult)
                t2 = initx.tile([128, 512], FP32, tag="tg")
                nc.scalar.activation(t2[:, 0:ncol], ps_g[2][:, 0:ncol],
                                     AF.Sigmoid, bias=bc_sb[:, 8 + f:9 + f])
                nc.gpsimd.tensor_tensor(v3(t2), v3(t2), cr, OP.mult)
                nc.gpsimd.tensor_tensor(cn, cn, v3(t2), OP.add)
                t3 = initx.tile([128, 512], FP32, tag="tg")
                nc.scalar.activation(t3[:, 0:ncol], ps_g[3][:, 0:ncol],
                                     AF.Tanh, bias=bc_sb[:, 12 + f:13 + f])
                nc.gpsimd.tensor_tensor(v3(t3), v3(t3), v3(si), OP.mult)
                nc.gpsimd.tensor_tensor(cn, cn, v3(t3), OP.add)
                so = initx.tile([128, 512], FP32, tag="so")
                nc.scalar.activation(so[:, 0:ncol], ps_g[4][:, 0:ncol],
                                     AF.Sigmoid, bias=bc_sb[:, 16 + f:17 + f])
                tcn = initx.tile([128, 512], FP32, tag="tg")
                nc.scalar.activation(v3(tcn), cn, AF.Tanh)
                nc.vector.tensor_tensor(hn, v3(so), v3(tcn), OP.mult)

        # ---- init logits ----
        nc.vector.memset(l_t[:], NEG)
        md31v = md31_sb[:].rearrange("p (s b) -> p s b", b=BC)
        for (sc0, sc1, bh) in chunks:
            ncol = (sc1 - sc0) * bh
            pl = inittr.tile([NP3, 512], FP32, tag="pl")
            chv = candh[:].rearrange("p f (s b) -> p f s b", b=BC)
            for f in range(4):
                nc.tensor.matmul(
                    pl[:, 0:ncol], qrep_sb[:, f, :],
                    chv[:, f, sc0:sc1, 0:bh],
                    start=(f == 0), stop=(f == 3))
            lm = initxb.tile([NP3, 512], FP32, tag="lm")
            nc.vector.tensor_tensor(
                lm[:, 0:ncol].rearrange("p (s b) -> p s b", b=bh),
                pl[:, 0:ncol].rearrange("p (s b) -> p s b", b=bh),
                md31v[:, sc0:sc1, 0:bh], OP.mult)
            nc.vector.tensor_reduce(
                l_t[:, sc0:sc1],
                lm[:, 0:ncol].rearrange("p (s b) -> p s b", b=bh),
                AX.X, OP.add)
        # mask invalid: l = l*lmask + (1-lmask)*NEG; (lmask-1)*(-NEG) is that term
        tmpl = initx.tile([NP3, 32], FP32, tag="tmpl")
        nc.vector.tensor_scalar(tmpl[:], lmask_sb[:], 1.0, -NEG, OP.subtract,
                                OP.mult)
        nc.vector.tensor_tensor(l_t[:], l_t[:], lmask_sb[:], OP.mult)
        nc.vector.tensor_tensor(l_t[:], l_t[:], tmpl[:], OP.add)

        # ---- write init candidate rows to table ((s,b) order) ----
        for blk in range(8):
            c0 = blk * 128
            c1 = min(c0 + 128, NTOT)
            w = c1 - c0
            bm = initxb.tile([128, 1024], FP32, tag="bm")
            for m in range(8):
                src = candh if m < 4 else candc
                pt = inittr.tile([128, 128], FP32, tag="pt")
                nc.tensor.transpose(
                    pt[0:w, :], src[:, m % 4, c0:c1], ident[:])
                nc.vector.tensor_copy(bm[0:w, m * 128:(m + 1) * 128], pt[0:w, :])
            s0 = 32 + 4 * blk
            s1 = 32 + min(4 * blk + 4, NIC)
            for j in range(s1 - s0):
                init_writes.append(qdma(
                    out=tabv_eb[s0 + j, 0:32, :],
                    in_=bm[32 * j:32 * j + 32, :]))

    # ================= iterations =================
    itp = ctx.enter_context(tc.tile_pool(name="itp", bufs=2))
    itk = ctx.enter_context(tc.tile_pool(name="itk", bufs=2))
    gps = ctx.enter_context(tc.tile_pool(name="gps", bufs=3, space="PSUM"))
    trs = ctx.enter_context(tc.tile_pool(name="trs", bufs=2, space="PSUM"))
    ops = ctx.enter_context(tc.tile_pool(name="ops", bufs=1, space="PSUM"))

    dbg = io.get("dbg")
    G = state.tile([NP3, 2 * H], FP32, tag="G")
    last_writes = init_writes[-3:]
    for i in range(imax):
        ab = min(int(sched[i]), 32)   # active sequences this iteration
        dcol = done_sb[:, i:i + 1]
        if dbg is not None:
            nc.sync.dma_start(out=dbg["l"][i], in_=l_t[:])

        # -- 1. argmax over logits --
        mx8 = itk.tile([NP3, 8], FP32, tag="mx8")
        nc.vector.max(mx8[:], l_t[:])
        ix8 = itk.tile([NP3, 8], U32, tag="ix8")
        nc.vector.max_index(ix8[:], mx8[:], l_t[:])
        sstar = itk.tile([NP3, 1], FP32, tag="sstar")
        nc.vector.tensor_copy(sstar[:], ix8[:, 0:1])

        # -- 2. linked-list row gathers --
        ohs = itk.tile([NP3, 32], FP32, tag="ohs")
        nc.vector.tensor_scalar(ohs[:], iota_sb[:], sstar[:], None,
                                OP.is_equal)
        tmp = itk.tile([NP3, 32], FP32, tag="tmp")

        def rowgather(arr, oh, name):
            col = itk.tile([NP3, 1], FP32, tag=name)
            nc.vector.tensor_tensor(tmp[:], oh, arr, OP.mult)
            nc.vector.tensor_reduce(col[:], tmp[:], AX.X, OP.max)
            return col

        nstar = rowgather(nxt_t[:], ohs[:], "nstar")
        pstar = rowgather(prv_t[:], ohs[:], "pstar")
        ci = rowgather(cidx_t[:], ohs[:], "ci")
        ohn = itk.tile([NP3, 32], FP32, tag="ohn")
        nc.vector.tensor_scalar(ohn[:], iota_sb[:], nstar[:], None,
                                OP.is_equal)
        n2 = rowgather(nxt_t[:], ohn[:], "n2")
        ohp = itk.tile([NP3, 32], FP32, tag="ohp")
        nc.vector.tensor_scalar(ohp[:], iota_sb[:], pstar[:], None,
                                OP.is_equal)
        e1c = rowgather(loc_t[:], ohp[:], "e1c")
        ohn2 = itk.tile([NP3, 32], FP32, tag="ohn2")
        nc.vector.tensor_scalar(ohn2[:], iota_sb[:], n2[:], None, OP.is_equal)
        e3c = rowgather(loc_t[:], ohn2[:], "e3c")

        # -- 3. gather index column [96,1]: groups (E1 | E2 | E3) + b*NE --
        gsel = itk.tile([NP3, 1], FP32, tag="gsel")
        nc.vector.tensor_copy(gsel[0:32, :], e1c[0:32, :])
        nc.vector.tensor_copy(gsel[32:64, :], ci[32:64, :])
        nc.vector.tensor_copy(gsel[64:96, :], e3c[64:96, :])
        nc.vector.tensor_tensor(gsel[:], gsel[:], bcol_sb[:], OP.add)
        gidx = itk.tile([NP3, 1], U32, tag="gidx")
        nc.vector.tensor_copy(gidx[:], gsel[:])
        if dbg is not None:
            nc.sync.dma_start(out=dbg["ss"][i], in_=sstar[:])
            nc.sync.dma_start(out=dbg["gs"][i], in_=gsel[:])

        # -- 4. indirect gather of entity rows (must order after table writes;
        #       Tile does not track RAW hazards through DRAM) --
        if False:
            for p0 in (0, 32, 64):
                gins = nc.gpsimd.indirect_dma_start(
                    out=G[p0:p0 + ab, :], out_offset=None, in_=tab,
                    in_offset=bass.IndirectOffsetOnAxis(
                        ap=gidx[p0:p0 + ab, :1], axis=0))
                for wr in last_writes:
                    add_dep_helper(gins.ins, wr.ins, reason="table RAW")
        else:
            gins = nc.gpsimd.indirect_dma_start(
                out=G[:], out_offset=None, in_=tab,
                in_offset=bass.IndirectOffsetOnAxis(ap=gidx[:, :1], axis=0))
            for wr in last_writes:
                add_dep_helper(gins.ins, wr.ins, reason="table RAW")

        # -- masks + pointer updates (independent of the compose results;
        #    run them here so they overlap the gather/matmul) --
        ohs_d = itk.tile([NP3, 32], FP32, tag="ohsd")
        nc.vector.tensor_scalar(ohs_d[:], iota_sb[:], sstar[:], dcol,
                                OP.is_equal, OP.mult)
        ohs_d8 = itk.tile([NP3, 32], U8, tag="ohsd8")
        nc.vector.tensor_copy(ohs_d8[:], ohs_d[:])
        m1 = itk.tile([NP3, 32], FP32, tag="m1")
        nc.vector.tensor_scalar(m1[:], iota_sb[:], pstar[:], dcol,
                                OP.is_equal, OP.mult)
        m1_8 = itk.tile([NP3, 32], U8, tag="m18")
        nc.vector.tensor_copy(m1_8[:], m1[:])
        ohn_d8 = itk.tile([NP3, 32], U8, tag="ohnd8")
        nc.vector.tensor_scalar(tmp[:], iota_sb[:], nstar[:], dcol,
                                OP.is_equal, OP.mult)
        nc.vector.tensor_copy(ohn_d8[:], tmp[:])
        ohn2_d8 = itk.tile([NP3, 32], U8, tag="ohn2d8")
        nc.vector.tensor_scalar(tmp[:], iota_sb[:], n2[:], dcol,
                                OP.is_equal, OP.mult)
        nc.vector.tensor_copy(ohn2_d8[:], tmp[:])
        ex2 = itk.tile([NP3, 1], FP32, tag="ex2")
        nc.vector.tensor_scalar(ex2[:], n2[:], SENT, None, OP.is_lt)

        # loc[s*] = ci ; nxt[s*] = n2 ; prv[n2] = s* ; cidx[p*/s*] = e1/e2
        e1v, e2v = float(63 + 2 * i), float(64 + 2 * i)
        nc.vector.copy_predicated(loc_t[:], ohs_d8[:],
                                  ci[:].to_broadcast([NP3, 32]))
        nc.vector.copy_predicated(nxt_t[:], ohs_d8[:],
                                  n2[:].to_broadcast([NP3, 32]))
        nc.vector.copy_predicated(prv_t[:], ohn2_d8[:],
                                  sstar[:].to_broadcast([NP3, 32]))
        cst = itk.tile([NP3, 1], FP32, tag="cst")
        nc.vector.memset(cst[:], e1v)
        nc.vector.copy_predicated(cidx_t[:], m1_8[:],
                                  cst[:].to_broadcast([NP3, 32]))
        cst2 = itk.tile([NP3, 1], FP32, tag="cst2")
        nc.vector.memset(cst2[:], e2v)
        nc.vector.copy_predicated(cidx_t[:], ohs_d8[:],
                                  cst2[:].to_broadcast([NP3, 32]))

        # -- 5. transpose to feature-major --
        Th = itp.tile([128, 4, NP3], FP32, tag="Th")
        Tc = itp.tile([128, 4, NP3], FP32, tag="Tc")
        for f in range(4):
            pt = trs.tile([128, 128], FP32, tag="pt")
            nc.tensor.transpose(pt[:, 0:NP3], G[:, f * 128:(f + 1) * 128],
                                ident[0:NP3, 0:NP3])
            nc.vector.tensor_copy(Th[:, f], pt[:, 0:NP3])
            pt2 = trs.tile([128, 128], FP32, tag="pt")
            nc.tensor.transpose(pt2[:, 0:NP3],
                                G[:, 512 + f * 128:512 + (f + 1) * 128],
                                ident[0:NP3, 0:NP3])
            nc.scalar.copy(Tc[:, f], pt2[:, 0:NP3])

        # -- 6. compose matmul for the 2 fresh pairs (cols: pair1 b | pair2 b) --
        # psum per gate holds all 4 f-tiles [128, (f,pair,b)=256]; bias folded
        # in via a K=1 matmul so one activation per gate covers all f.
        hcand = itp.tile([128, 4, 64], FP32, tag="hcand")
        ccand = itp.tile([128, 4, 64], FP32, tag="ccand")
        # active sequences are a sorted prefix: pair1 occupies cols [0:32],
        # pair2's active lanes [32:32+ab] -> one contiguous N=32+ab slice.
        nmm = 32 + ab
        acts = []
        for g in range(5):
            ps = gps.tile([128, 512], FP32, tag="pg")
            for f in range(4):
                mt = g * 4 + f
                nc.tensor.matmul(
                    ps[:, f * 64:f * 64 + 64],
                    bcrow_sb[0:1, mt * 128:(mt + 1) * 128],
                    ones_sb[0:1, 0:64], start=True, stop=False)
                for kt in range(8):
                    rv = (Th[:, kt, 0:nmm] if kt < 4
                          else Th[:, kt - 4, 32:32 + nmm])
                    nc.tensor.matmul(
                        ps[:, f * 64:f * 64 + nmm],
                        wc_sb[:, kt, mt * 128:(mt + 1) * 128],
                        rv, start=False, stop=(kt == 7))
            a = itk.tile([128, 256], FP32, tag=f"ga{g}")
            nc.scalar.activation(a[:], ps[:, 0:256],
                                 AF.Tanh if g == 3 else AF.Sigmoid)
            acts.append(a)
        si, t1, t2, t3, so = acts

        def a3(t):
            return t[:].rearrange("p (f n) -> p f n", n=64)

        cl = Tc[:, :, 0:64]
        cr = Tc[:, :, 32:96]
        cn = ccand[:]
        hn = hcand[:]
        nc.vector.tensor_tensor(cn, a3(t1), cl, OP.mult)
        nc.gpsimd.tensor_tensor(a3(t2), a3(t2), cr, OP.mult)
        nc.gpsimd.tensor_tensor(cn, cn, a3(t2), OP.add)
        nc.gpsimd.tensor_tensor(a3(t3), a3(t3), a3(si), OP.mult)
        nc.vector.tensor_tensor(cn, cn, a3(t3), OP.add)
        tcn = itk.tile([128, 256], FP32, tag="tg")
        nc.scalar.activation(a3(tcn), cn, AF.Tanh)
        nc.vector.tensor_tensor(hn, a3(so), a3(tcn), OP.mult)

        # -- 7. fresh logits --
        pl = ops.tile([NP3, 64], FP32, tag="pl")
        for f in range(4):
            nc.tensor.matmul(pl[:], qrep_sb[:, f, :], hcand[:, f, :],
                             start=(f == 0), stop=(f == 3))
        lmt = itk.tile([NP3, 64], FP32, tag="lmt")
        nc.vector.tensor_tensor(lmt[:], pl[:], md2_sb[:], OP.mult)
        lnew = itk.tile([NP3, 2], FP32, tag="lnew")
        nc.vector.tensor_reduce(
            lnew[:], lmt[:].rearrange("p (r b) -> p r b", r=2), AX.X, OP.add)
        if dbg is not None:
            nc.sync.dma_start(out=dbg["ln"][i], in_=lnew[:])

        # -- 8. write fresh candidate rows to table --
        W = itp.tile([64, 8, 128], FP32, tag="W")
        for f in range(4):
            po = trs.tile([128, 128], FP32, tag="pt")
            nc.tensor.transpose(po[0:64, :], hcand[:, f, :], ident[:])
            nc.vector.tensor_copy(W[:, f], po[0:64, :])
            po2 = trs.tile([128, 128], FP32, tag="pt")
            nc.tensor.transpose(po2[0:64, :], ccand[:, f, :], ident[:])
            nc.vector.tensor_copy(W[:, 4 + f], po2[0:64, :])
        w1 = qdma(out=tabv_eb[63 + 2 * i, 0:ab, :],
                  in_=W[0:ab].rearrange("p m d -> p (m d)"))
        w2 = qdma(out=tabv_eb[64 + 2 * i, 0:ab, :],
                  in_=W[32:32 + ab].rearrange("p m d -> p (m d)"))
        last_writes = [w1, w2]

        # -- 9. logit updates (only remaining post-compose bookkeeping) --
        # l[p*] = v1 ; l[s*] = v2 if n2 exists else NEG ; l[n*] = NEG
        nc.vector.copy_predicated(l_t[:], m1_8[:],
                                  lnew[:, 0:1].to_broadcast([NP3, 32]))
        # v2p = v2*ex2 + NEG*(1-ex2), avoiding 1e9 absorption of v2
        v2p = itk.tile([NP3, 1], FP32, tag="v2p")
        nc.vector.tensor_tensor(v2p[:], lnew[:, 1:2], ex2[:], OP.mult)
        negpart = itk.tile([NP3, 1], FP32, tag="negpart")
        nc.vector.tensor_scalar(negpart[:], ex2[:], 1.0, -NEG, OP.subtract,
                                OP.mult)
        nc.vector.tensor_tensor(v2p[:], v2p[:], negpart[:], OP.add)
        nc.vector.copy_predicated(l_t[:], ohs_d8[:],
                                  v2p[:].to_broadcast([NP3, 32]))
        nc.vector.copy_predicated(l_t[:], ohn_d8[:],
                                  negc_sb[:].to_broadcast([NP3, 32]))

    # ================= output =================
    oidx = itk.tile([BC, 1], FP32, tag="oidx")
    nc.vector.tensor_tensor(oidx[:], loc_t[0:BC, 0:1], bcol_sb[0:BC, :],
                            OP.add)
    oidxu = itk.tile([BC, 1], U32, tag="oidxu")
    nc.vector.tensor_copy(oidxu[:], oidx[:])
    Gout = itp.tile([BC, 2 * H], FP32, tag="Gout")
    gout_ins = nc.gpsimd.indirect_dma_start(
        out=Gout[:], out_offset=None, in_=tab,
        in_offset=bass.IndirectOffsetOnAxis(ap=oidxu[:, :1], axis=0))
    for wr in last_writes:
        add_dep_helper(gout_ins.ins, wr.ins, reason="table RAW")
    nc.sync.dma_start(out=out_d, in_=Gout[:, 0:H])


_BUILD_CACHE = {}


def build(sched=None):
    if sched is None:
        sched = (32,) * NIC
    sched = tuple(int(v) for v in sched)
    if sched in _BUILD_CACHE:
        return _BUILD_CACHE[sched]
    nc = bacc.Bacc("TRN2", target_bir_lowering=False, debug=False)
    io = {
        "xT": nc.dram_tensor("xT", [4, 128, BC * L], FP32, kind="ExternalInput").ap(),
        "wwT": nc.dram_tensor("wwT", [4, 128, 2 * H], FP32, kind="ExternalInput").ap(),
        "wcT": nc.dram_tensor("wcT", [8, 128, 5 * H], FP32, kind="ExternalInput").ap(),
        "bw": nc.dram_tensor("bw", [128, 8], FP32, kind="ExternalInput").ap(),
        "bc": nc.dram_tensor("bc", [128, 20], FP32, kind="ExternalInput").ap(),
        "bcrow": nc.dram_tensor("bcrow", [1, 5 * H], FP32, kind="ExternalInput").ap(),
        "qrep": nc.dram_tensor("qrep", [128, 4, 96], FP32, kind="ExternalInput").ap(),
        "tab": nc.dram_tensor("tab", [BC * NE, 2 * H], FP32, kind="ExternalInput").ap(),
        "iota32": nc.dram_tensor("iota32", [NP3, 32], FP32, kind="ExternalInput").ap(),
        "bcol": nc.dram_tensor("bcol", [NP3, 1], FP32, kind="ExternalInput").ap(),
        "mdiag2": nc.dram_tensor("mdiag2", [NP3, 64], FP32, kind="ExternalInput").ap(),
        "mdiag31": nc.dram_tensor("mdiag31", [NP3, NIC * BC], FP32, kind="ExternalInput").ap(),
        "lmask": nc.dram_tensor("lmask", [NP3, 32], FP32, kind="ExternalInput").ap(),
        "nxt0": nc.dram_tensor("nxt0", [NP3, 32], FP32, kind="ExternalInput").ap(),
        "prv0": nc.dram_tensor("prv0", [NP3, 32], FP32, kind="ExternalInput").ap(),
        "done": nc.dram_tensor("done", [NP3, NIC], FP32, kind="ExternalInput").ap(),
        "out": nc.dram_tensor("out", [BC, H], FP32, kind="ExternalOutput").ap(),
    }
    if DEBUG:
        io["dbg"] = {
            "l": nc.dram_tensor("dbg_l", [NIC, NP3, 32], FP32, kind="ExternalOutput").ap(),
            "ss": nc.dram_tensor("dbg_ss", [NIC, NP3, 1], FP32, kind="ExternalOutput").ap(),
            "gs": nc.dram_tensor("dbg_gs", [NIC, NP3, 1], FP32, kind="ExternalOutput").ap(),
            "ln": nc.dram_tensor("dbg_ln", [NIC, NP3, 2], FP32, kind="ExternalOutput").ap(),
        }
    with tile.TileContext(nc) as tc:
        with ExitStack() as ctx:
            build_kernel(ctx, tc, io, sched)
    nc.compile()
    _BUILD_CACHE[sched] = nc
    return nc


def make_sched(length):
    length = np.asarray(length).astype(np.int64)
    cnt = [(length > i + 1).sum() for i in range(NIC)]
    return tuple(int(-(-c // NCORES)) for c in cnt)


def make_order(length):
    length = np.asarray(length).astype(np.int64)
    order = np.argsort(-length, kind="stable")
    return order.reshape(L, NCORES)


def make_in_maps(x, length, W_word, b_word, W_comp, b_comp, comp_query):
    x = np.asarray(x, np.float32)
    length = np.asarray(length).astype(np.int64)
    W_word = np.asarray(W_word, np.float32)
    b_word = np.asarray(b_word, np.float32)
    W_comp = np.asarray(W_comp, np.float32)
    b_comp = np.asarray(b_comp, np.float32)
    comp_query = np.asarray(comp_query, np.float32)

    wwT = np.ascontiguousarray(W_word.T.reshape(4, 128, 2 * H))
    wcT = np.ascontiguousarray(W_comp.T.reshape(8, 128, 5 * H))
    bw = np.ascontiguousarray(b_word.reshape(8, 128).T)
    bca = b_comp.copy()
    bca[H:3 * H] += 1.0
    bc = np.ascontiguousarray(bca.reshape(20, 128).T)
    qs = (comp_query * (1.0 / np.sqrt(H))).astype(np.float32)
    qrep = np.ascontiguousarray(
        np.broadcast_to(qs.reshape(4, 128, 1), (4, 128, NP3))
        .transpose(1, 0, 2)).astype(np.float32)

    iota32 = np.tile(np.arange(32, dtype=np.float32), (NP3, 1))
    bcol = (np.arange(NP3, dtype=np.float32) % BC).reshape(NP3, 1) * NE
    bidx = np.arange(NP3) % BC
    md2 = np.zeros((NP3, 64), np.float32)
    md2[np.arange(NP3), bidx] = 1.0
    md2[np.arange(NP3), 32 + bidx] = 1.0
    md31 = np.zeros((NP3, NIC, BC), np.float32)
    md31[np.arange(NP3), :, bidx] = 1.0
    md31 = md31.reshape(NP3, NIC * BC)
    tabz = np.zeros((BC * NE, 2 * H), np.float32)

    ordmat = make_order(length)
    in_maps = []
    for k in range(NCORES):
        idxs = ordmat[:, k]
        xs = x[idxs]
        xT = np.ascontiguousarray(xs.transpose(2, 0, 1).reshape(4, 128, BC * L))
        ln = length[idxs].astype(np.int64)
        lnr = ln[bidx]  # [96]
        lmask = (np.arange(32)[None, :] < (lnr[:, None] - 1)).astype(np.float32)
        lmask[:, 31] = 0.0
        nxt0 = np.full((NP3, 32), SENT, np.float32)
        prv0 = np.full((NP3, 32), SENT, np.float32)
        for p in range(NP3):
            m = int(lnr[p])
            for s in range(m - 1):
                nxt0[p, s] = s + 1
            for s in range(1, m):
                prv0[p, s] = s - 1
        done = (np.arange(1, L)[None, :] < lnr[:, None]).astype(np.float32)
        in_maps.append({
            "xT": xT, "wwT": wwT, "wcT": wcT, "bw": bw, "bc": bc,
            "bcrow": np.ascontiguousarray(bca.reshape(1, 5 * H)),
            "qrep": qrep, "tab": tabz, "iota32": iota32, "bcol": bcol,
            "mdiag2": md2, "mdiag31": md31, "lmask": lmask,
            "nxt0": nxt0, "prv0": prv0, "done": done,
        })
    return in_maps


def kernel(x, length, W_word, b_word, W_comp, b_comp, comp_query):
    nc = build(make_sched(length))
    in_maps = make_in_maps(x, length, W_word, b_word, W_comp, b_comp, comp_query)
    res = run_bass_kernel_spmd(nc, in_maps, list(range(NCORES)))
    out = np.zeros((B, H), np.float32)
    ordmat = make_order(length)
    for k in range(NCORES):
        out[ordmat[:, k]] = res.results[k]["out"]
    return out



# revision 46
# speedup vs baseline: 4.0477x; 3.8213x over previous
"""Bass/Tile TRN2 kernel for nn_BinaryTreeLSTM (B=256, L=32, D=512, H=512).

Incremental greedy TreeLSTM, data-parallel over batch (32 seqs/core, 8 cores).

Instead of recomputing all adjacent-pair compositions each of the 31 shrink
iterations (O(L^2) matmul work), maintains per-sequence state in "slot space"
(no physical shifting) plus an entity table of h|c value rows in DRAM:
  rows b*128+e: e in [0,32) leaf states, [32,63) initial pair candidates,
  [63,125) fresh candidates (2 per iteration).
Each iteration: argmax over maintained logits -> linked-list bookkeeping row
ops -> one indirect-DMA gather of 3 entity rows per sequence -> PE transposes
to feature-major -> compose matmul for just the 2 fresh pairs -> gating ->
fresh logits + masked scatter updates -> table write. All fp32 (greedy argmax
selection is precision-sensitive; bf16/tf32 matmuls flip merge decisions).
"""

import math
import sys
import numpy as np

sys.path.insert(0, "/opt/trn_rl_repo")

from contextlib import ExitStack

import concourse.bass as bass
import concourse.tile as tile
from concourse import bacc, mybir
from concourse.bass_utils import run_bass_kernel_spmd
from concourse.masks import make_identity
from concourse.tile import add_dep_helper

FP32 = mybir.dt.float32
BF16 = mybir.dt.bfloat16
U32 = mybir.dt.uint32
U8 = mybir.dt.uint8

B, L, D, H = 256, 32, 512, 512
NCORES = 8
BC = B // NCORES          # 32 sequences per core
NE = 128                  # entity rows per sequence in the table
NEG = -1.0e9
SENT = 33.0
NP3 = 64                  # 2 replicated bookkeeping groups of 32 partitions
AF = mybir.ActivationFunctionType
OP = mybir.AluOpType
AX = mybir.AxisListType
NIC = L - 1               # 31 initial candidates / iterations
DEBUG = False


def build_kernel(ctx: ExitStack, tc: "tile.TileContext", io: dict, sched,
                 max_iters=None, ablate=()):
    # ablate: timing-only experiment knobs ("compose", "dma", "book", "acts")
    nc = tc.nc
    imax = max(i for i in range(NIC) if sched[i] >= 1) + 1
    if max_iters is not None:
        imax = min(imax, max_iters)

    # rotate bulk DMAs across engine queues; gathers get explicit deps
    dmaq = [nc.sync, nc.scalar]
    qi = [0]

    def qdma(**kw):
        eng = dmaq[qi[0] % 2]
        qi[0] += 1
        return eng.dma_start(**kw)

    xT = io["xT"]          # [4, 128, BC*L]
    wwT = io["wwT"]        # [4, 128, 2H]
    wcT = io["wcT"]        # [8, 128, 5H]
    bw_d = io["bw"]        # [128, 8]
    bc_d = io["bc"]        # [128, 20]  (+1.0 folded into fl/fr)
    qrep_d = io["qrep"]    # [128, 4, 96] scaled comp_query replicated over M
    tab = io["tab"]        # [BC*NE, 2H] value table (zero-filled)
    iota_d = io["iota32"]  # [96, 32]
    bcol_d = io["bcol"]    # [96, 1]  (p%32)*NE
    md2_d = io["mdiag2"]   # [96, 64]
    md31_d = io["mdiag31"] # [96, 992]
    lmask_d = io["lmask"]  # [96, 32]
    nxt0_d = io["nxt0"]    # [96, 32]
    prv0_d = io["prv0"]    # [96, 32]
    done_d = io["done"]    # [96, 31]
    out_d = io["out"]      # [BC, H]

    tabv_be = tab.rearrange("(b e) d -> b e d", b=BC)   # [32, 128, 1024]
    tabv_eb = tab.rearrange("(b e) d -> e b d", b=BC)   # [128, 32, 1024]

    consts = ctx.enter_context(tc.tile_pool(name="consts", bufs=1))
    state = ctx.enter_context(tc.tile_pool(name="state", bufs=1))

    # ---- persistent constants ----
    bw_sb = consts.tile([128, 8], FP32, tag="bw")
    nc.sync.dma_start(out=bw_sb[:], in_=bw_d[:])
    bc_sb = consts.tile([128, 20], FP32, tag="bc")
    nc.sync.dma_start(out=bc_sb[:], in_=bc_d[:])
    qrep_sb = consts.tile([128, 4, NP3], FP32, tag="qrep")
    nc.sync.dma_start(out=qrep_sb[:], in_=qrep_d[:])
    iota_sb = consts.tile([NP3, 32], FP32, tag="iota")
    nc.sync.dma_start(out=iota_sb[:], in_=iota_d[:])
    bcol_sb = consts.tile([NP3, 1], FP32, tag="bcol")
    nc.sync.dma_start(out=bcol_sb[:], in_=bcol_d[:])
    md2_sb = consts.tile([NP3, 64], FP32, tag="md2")
    nc.sync.dma_start(out=md2_sb[:], in_=md2_d[:])
    lmask_sb = consts.tile([NP3, 32], FP32, tag="lmask")
    nc.sync.dma_start(out=lmask_sb[:], in_=lmask_d[:])
    done_sb = consts.tile([NP3, NIC], FP32, tag="done")
    nc.sync.dma_start(out=done_sb[:], in_=done_d[:])
    negc_sb = consts.tile([NP3, 1], FP32, tag="negc")
    nc.vector.memset(negc_sb[:], NEG)
    bcrow_sb = consts.tile([1, 5 * H], FP32, tag="bcrow")
    nc.sync.dma_start(out=bcrow_sb[:], in_=io["bcrow"])
    ones_sb = consts.tile([1, 96], FP32, tag="ones")
    nc.vector.memset(ones_sb[:], 1.0)
    # bias selector for the packed compose: 1 on hi rows (m<64), 0 on lo rows
    bstat_sb = consts.tile([1, 128], FP32, tag="bstat")
    nc.vector.memset(bstat_sb[:, 0:64], 1.0)
    nc.vector.memset(bstat_sb[:, 64:128], 0.0)
    ident = consts.tile([128, 128], FP32, tag="ident")
    make_identity(nc, ident[:])

    # ---- bookkeeping state (3 replicated groups of 32 partitions) ----
    l_t = state.tile([NP3, 32], FP32, tag="l")
    nxt_t = state.tile([NP3, 32], FP32, tag="nxt")
    nc.sync.dma_start(out=nxt_t[:], in_=nxt0_d[:])
    prv_t = state.tile([NP3, 32], FP32, tag="prv")
    nc.sync.dma_start(out=prv_t[:], in_=prv0_d[:])
    loc_t = state.tile([NP3, 32], FP32, tag="loc")
    nc.vector.tensor_copy(loc_t[:], iota_sb[:])
    cidx_t = state.tile([NP3, 32], FP32, tag="cidx")
    nc.vector.tensor_scalar(cidx_t[:], iota_sb[:], 32.0, None, OP.add)
    nc.vector.memset(cidx_t[:, 31:32], 0.0)

    # ================= init phase =================
    with tc.tile_pool(name="initp", bufs=1) as initp, \
         tc.tile_pool(name="initx", bufs=2) as initx, \
         tc.tile_pool(name="initxb", bufs=1) as initxb, \
         tc.tile_pool(name="initps", bufs=5, space="PSUM") as initps, \
         tc.tile_pool(name="inittr", bufs=1, space="PSUM") as inittr:

        wc_sb = initp.tile([128, 8, 5 * H], FP32, tag="wc")
        for kt in range(8):
            nc.sync.dma_start(out=wc_sb[:, kt, :], in_=wcT[kt])
        ww_sb = initp.tile([128, 4, 2 * H], FP32, tag="ww")
        for kt in range(4):
            nc.sync.dma_start(out=ww_sb[:, kt, :], in_=wwT[kt])
        md31_sb = initp.tile([NP3, 992], FP32, tag="md31")
        nc.sync.dma_start(out=md31_sb[:], in_=md31_d[:])

        # ---- leaves: hc = W_word @ x ; layout [128, f, b, s] ----
        hleaf = initp.tile([128, 4, BC, L], FP32, tag="hleaf")
        cleaf = initp.tile([128, 4, BC, L], FP32, tag="cleaf")
        if "init_leafmm" in ablate:
            nc.vector.memset(hleaf[:], 0.1)
            nc.vector.memset(cleaf[:], 0.1)
        else:
            for cidx2 in range(2):
                xb = []
                for kt in range(4):
                    xt = initxb.tile([128, 512], FP32, tag=f"xb{kt}")
                    nc.sync.dma_start(
                        out=xt[:], in_=xT[kt][:, cidx2 * 512:(cidx2 + 1) * 512])
                    xb.append(xt)
                for m in range(8):
                    ps = initps.tile([128, 512], FP32, tag="pg")
                    for kt in range(4):
                        nc.tensor.matmul(
                            ps[:], ww_sb[:, kt, m * 128:(m + 1) * 128],
                            xb[kt][:], start=(kt == 0), stop=(kt == 3))
                    dst = hleaf if m < 4 else cleaf
                    dview = dst[:, m % 4, 16 * cidx2:16 * cidx2 + 16, :]
                    nc.vector.tensor_scalar(
                        dview, ps[:].rearrange("p (b n) -> p b n", b=16),
                        bw_sb[:, m:m + 1], None, OP.add)

        # ---- write leaf rows to table (b-major) ----
        # per-b 2D DMAs: 3D APs (and degenerate [1,1] dims from rearrange)
        # shatter descriptor merging and cost ~30x on the DMA engines.
        init_writes = []
        if "init_leafwr" not in ablate:
            for blk in range(8):
                bm = initxb.tile([128, 1024], FP32, tag="bm")
                for m in range(8):
                    src = hleaf if m < 4 else cleaf
                    sv = src[:, m % 4].rearrange("p b s -> p (b s)")
                    pt = inittr.tile([128, 128], FP32, tag="pt")
                    nc.tensor.transpose(
                        pt[:], sv[:, blk * 128:(blk + 1) * 128], ident[:])
                    nc.vector.tensor_copy(bm[:, m * 128:(m + 1) * 128], pt[:])
                for j in range(4):
                    init_writes.append(qdma(
                        out=tabv_be[4 * blk + j, 0:32, :],
                        in_=bm[32 * j:32 * j + 32, :]))

        # ---- initial candidates: compose valid adjacent pairs, (s,b) order ----
        candh = initp.tile([128, 4, NIC * BC], FP32, tag="candh")
        candc = initp.tile([128, 4, NIC * BC], FP32, tag="candc")
        nc.vector.memset(candh[:], 0.0)
        nc.vector.memset(candc[:], 0.0)

        def leaf_v(t, kt, s0, s1, bh):
            return t[:, kt].rearrange("p b s -> p s b")[:, s0:s1, 0:bh]

        NTOT = NIC * BC  # 992
        # s-chunks with per-chunk active-b bound from the baked schedule
        chunks = []
        s0 = 0
        while s0 < imax:
            bh = max(sched[s0], 1)
            s1 = s0 + 1
            while s1 < imax and (s1 + 1 - s0) * bh <= 512:
                s1 += 1
            chunks.append((s0, s1, bh))
            s0 = s1
        for (sc0, sc1, bh) in (() if "init_cand" in ablate else chunks):
            ncol = (sc1 - sc0) * bh
            for f in range(4):
                ps_g = []
                for g in range(5):
                    mt = g * 4 + f
                    ps = initps.tile([128, 512], FP32, tag="pg")
                    for kt in range(8):
                        if kt < 4:
                            rv = leaf_v(hleaf, kt, sc0, sc1, bh)
                        else:
                            rv = leaf_v(hleaf, kt - 4, sc0 + 1, sc1 + 1, bh)
                        nc.tensor.matmul(
                            ps[:, 0:ncol],
                            wc_sb[:, kt, mt * 128:(mt + 1) * 128], rv,
                            start=(kt == 0), stop=(kt == 7))
                    ps_g.append(ps)
                cl = leaf_v(cleaf, f, sc0, sc1, bh)
                cr = leaf_v(cleaf, f, sc0 + 1, sc1 + 1, bh)
                cn = candc[:, f].rearrange("p (s b) -> p s b", b=BC)[:, sc0:sc1, 0:bh]
                hn = candh[:, f].rearrange("p (s b) -> p s b", b=BC)[:, sc0:sc1, 0:bh]
                def v3(t):
                    return t[:, 0:ncol].rearrange("p (s b) -> p s b", b=bh)

                si = initx.tile([128, 512], FP32, tag="si")
                nc.scalar.activation(si[:, 0:ncol], ps_g[0][:, 0:ncol],
                                     AF.Sigmoid, bias=bc_sb[:, f:f + 1])
                t1 = initx.tile([128, 512], FP32, tag="tg")
                nc.scalar.activation(t1[:, 0:ncol], ps_g[1][:, 0:ncol],
                                     AF.Sigmoid, bias=bc_sb[:, 4 + f:5 + f])
                nc.vector.tensor_tensor(cn, v3(t1), cl, OP.mult)
                t2 = initx.tile([128, 512], FP32, tag="tg")
                nc.scalar.activation(t2[:, 0:ncol], ps_g[2][:, 0:ncol],
                                     AF.Sigmoid, bias=bc_sb[:, 8 + f:9 + f])
                nc.gpsimd.tensor_tensor(v3(t2), v3(t2), cr, OP.mult)
                nc.gpsimd.tensor_tensor(cn, cn, v3(t2), OP.add)
                t3 = initx.tile([128, 512], FP32, tag="tg")
                nc.scalar.activation(t3[:, 0:ncol], ps_g[3][:, 0:ncol],
                                     AF.Tanh, bias=bc_sb[:, 12 + f:13 + f])
                nc.gpsimd.tensor_tensor(v3(t3), v3(t3), v3(si), OP.mult)
                nc.gpsimd.tensor_tensor(cn, cn, v3(t3), OP.add)
                so = initx.tile([128, 512], FP32, tag="so")
                nc.scalar.activation(so[:, 0:ncol], ps_g[4][:, 0:ncol],
                                     AF.Sigmoid, bias=bc_sb[:, 16 + f:17 + f])
                tcn = initx.tile([128, 512], FP32, tag="tg")
                nc.scalar.activation(v3(tcn), cn, AF.Tanh)
                nc.vector.tensor_tensor(hn, v3(so), v3(tcn), OP.mult)

        # ---- init logits ----
        nc.vector.memset(l_t[:], NEG)
        md31v = md31_sb[:].rearrange("p (s b) -> p s b", b=BC)
        for (sc0, sc1, bh) in (() if "init_logit" in ablate else chunks):
            ncol = (sc1 - sc0) * bh
            pl = inittr.tile([NP3, 512], FP32, tag="pl")
            chv = candh[:].rearrange("p f (s b) -> p f s b", b=BC)
            for f in range(4):
                nc.tensor.matmul(
                    pl[:, 0:ncol], qrep_sb[:, f, :],
                    chv[:, f, sc0:sc1, 0:bh],
                    start=(f == 0), stop=(f == 3))
            lm = initxb.tile([NP3, 512], FP32, tag="lm")
            nc.vector.tensor_tensor(
                lm[:, 0:ncol].rearrange("p (s b) -> p s b", b=bh),
                pl[:, 0:ncol].rearrange("p (s b) -> p s b", b=bh),
                md31v[:, sc0:sc1, 0:bh], OP.mult)
            nc.vector.tensor_reduce(
                l_t[:, sc0:sc1],
                lm[:, 0:ncol].rearrange("p (s b) -> p s b", b=bh),
                AX.X, OP.add)
        # mask invalid: l = l*lmask + (1-lmask)*NEG; (lmask-1)*(-NEG) is that term
        tmpl = initx.tile([NP3, 32], FP32, tag="tmpl")
        nc.vector.tensor_scalar(tmpl[:], lmask_sb[:], 1.0, -NEG, OP.subtract,
                                OP.mult)
        nc.vector.tensor_tensor(l_t[:], l_t[:], lmask_sb[:], OP.mult)
        nc.vector.tensor_tensor(l_t[:], l_t[:], tmpl[:], OP.add)

        # ---- write init candidate rows to table ((s,b) order) ----
        for blk in (() if "init_candwr" in ablate else range(8)):
            c0 = blk * 128
            c1 = min(c0 + 128, NTOT)
            w = c1 - c0
            bm = initxb.tile([128, 1024], FP32, tag="bm")
            for m in range(8):
                src = candh if m < 4 else candc
                pt = inittr.tile([128, 128], FP32, tag="pt")
                nc.tensor.transpose(
                    pt[0:w, :], src[:, m % 4, c0:c1], ident[:])
                nc.vector.tensor_copy(bm[0:w, m * 128:(m + 1) * 128], pt[0:w, :])
            s0 = 32 + 4 * blk
            s1 = 32 + min(4 * blk + 4, NIC)
            for j in range(s1 - s0):
                init_writes.append(qdma(
                    out=tabv_eb[s0 + j, 0:32, :],
                    in_=bm[32 * j:32 * j + 32, :]))

    # ================= iterations =================
    itw = ctx.enter_context(tc.tile_pool(name="itw", bufs=1))
    itp = ctx.enter_context(tc.tile_pool(name="itp", bufs=2))
    itk = ctx.enter_context(tc.tile_pool(name="itk", bufs=2))
    gps = ctx.enter_context(tc.tile_pool(name="gps", bufs=1, space="PSUM"))
    trs = ctx.enter_context(tc.tile_pool(name="trs", bufs=2, space="PSUM"))
    ops = ctx.enter_context(tc.tile_pool(name="ops", bufs=1, space="PSUM"))

    # bf16 hi/lo split of W_comp^T for the packed compose (loaded after the
    # init pools release their SBUF)
    wcbh_sb = itw.tile([128, 8, 5 * H], BF16, tag="wcbh")
    wcbl_sb = itw.tile([128, 8, 5 * H], BF16, tag="wcbl")
    for kt in range(8):
        qdma(out=wcbh_sb[:, kt, :], in_=io["wcbh"][kt])
        qdma(out=wcbl_sb[:, kt, :], in_=io["wcbl"][kt])
    qrow_sb = itw.tile([64, H], FP32, tag="qrow")
    nc.sync.dma_start(out=qrow_sb[:], in_=io["qrow"])

    G = itw.tile([64, 2 * H], FP32, tag="G")      # rows [E1(b) | E2(b)]
    G2 = itw.tile([64, 2 * H], FP32, tag="G2")    # rows [E2(b) | E3(b)]
    # --- ablation support (timing-only experiment builds) ---
    if "dma" in ablate:
        nc.vector.memset(G[:], 0.3)
        nc.vector.memset(G2[:], 0.3)
    if "book" in ablate:
        abl_gidx = itw.tile([64, 1], U32, tag="abl_gidx")
        nc.vector.tensor_copy(abl_gidx[:], bcol_sb[0:64, :])
    if "compose" in ablate:
        abl_ps = ctx.enter_context(
            tc.tile_pool(name="ablps", bufs=1, space="PSUM"))
        abl_psd = abl_ps.tile([128, 512], FP32, tag="abl_psd")
        nc.vector.memset(abl_psd[:], 0.1)
    if "acts" in ablate:
        abl_lnew = itw.tile([NP3, 2], FP32, tag="abl_lnew")
        nc.vector.memset(abl_lnew[:], 0.0)

    dbg = io.get("dbg")
    last_writes = init_writes[-3:]
    for i in range(imax):
        ab = min(int(sched[i]), 32)   # active sequences this iteration
        dcol = done_sb[:, i:i + 1]
        if dbg is not None:
            nc.sync.dma_start(out=dbg["l"][i], in_=l_t[:])

        do_book = "book" not in ablate
        do_dma = "dma" not in ablate
        do_compose = "compose" not in ablate
        do_acts = "acts" not in ablate

        if do_book:
            # -- 1. argmax over logits --
            mx8 = itk.tile([NP3, 8], FP32, tag="mx8")
            nc.vector.max(mx8[:], l_t[:])
            ix8 = itk.tile([NP3, 8], U32, tag="ix8")
            nc.vector.max_index(ix8[:], mx8[:], l_t[:])
            sstar = itk.tile([NP3, 1], FP32, tag="sstar")
            nc.vector.tensor_copy(sstar[:], ix8[:, 0:1])

            # -- 2. linked-list row gathers --
            ohs = itk.tile([NP3, 32], FP32, tag="ohs")
            nc.vector.tensor_scalar(ohs[:], iota_sb[:], sstar[:], None,
                                    OP.is_equal)
            tmp = itk.tile([NP3, 32], FP32, tag="tmp")

            def rowgather(arr, oh, name):
                col = itk.tile([NP3, 1], FP32, tag=name)
                nc.vector.tensor_tensor(tmp[:], oh, arr, OP.mult)
                nc.vector.tensor_reduce(col[:], tmp[:], AX.X, OP.max)
                return col

            nstar = rowgather(nxt_t[:], ohs[:], "nstar")
            pstar = rowgather(prv_t[:], ohs[:], "pstar")
            ci = rowgather(cidx_t[:], ohs[:], "ci")
            ohn = itk.tile([NP3, 32], FP32, tag="ohn")
            nc.vector.tensor_scalar(ohn[:], iota_sb[:], nstar[:], None,
                                    OP.is_equal)
            n2 = rowgather(nxt_t[:], ohn[:], "n2")
            ohp = itk.tile([NP3, 32], FP32, tag="ohp")
            nc.vector.tensor_scalar(ohp[:], iota_sb[:], pstar[:], None,
                                    OP.is_equal)
            e1c = rowgather(loc_t[:], ohp[:], "e1c")
            ohn2 = itk.tile([NP3, 32], FP32, tag="ohn2")
            nc.vector.tensor_scalar(ohn2[:], iota_sb[:], n2[:], None,
                                    OP.is_equal)
            e3c = rowgather(loc_t[:], ohn2[:], "e3c")

            # -- 3. gather index columns [64,1]: (E1|E2) and (E2|E3) + b*NE --
            gsel = itk.tile([64, 1], FP32, tag="gsel")
            nc.vector.tensor_copy(gsel[0:32, :], e1c[0:32, :])
            nc.vector.tensor_copy(gsel[32:64, :], ci[32:64, :])
            nc.vector.tensor_tensor(gsel[:], gsel[:], bcol_sb[0:64, :],
                                    OP.add)
            gidx = itk.tile([64, 1], U32, tag="gidx")
            nc.vector.tensor_copy(gidx[:], gsel[:])
            gsel2 = itk.tile([64, 1], FP32, tag="gsel2")
            nc.vector.tensor_copy(gsel2[0:32, :], ci[0:32, :])
            nc.vector.tensor_copy(gsel2[32:64, :], e3c[32:64, :])
            nc.vector.tensor_tensor(gsel2[:], gsel2[:], bcol_sb[0:64, :],
                                    OP.add)
            gidx2 = itk.tile([64, 1], U32, tag="gidx2")
            nc.vector.tensor_copy(gidx2[:], gsel2[:])
            if dbg is not None:
                nc.sync.dma_start(out=dbg["ss"][i], in_=sstar[:])
                nc.sync.dma_start(out=dbg["gs"][i], in_=gsel[:])
        else:
            gidx = abl_gidx
            gidx2 = abl_gidx

        # -- 4. indirect gather of entity rows (must order after table writes;
        #       Tile does not track RAW hazards through DRAM) --
        if do_dma:
            gins = nc.gpsimd.indirect_dma_start(
                out=G[:], out_offset=None, in_=tab,
                in_offset=bass.IndirectOffsetOnAxis(ap=gidx[:, :1], axis=0))
            gins2 = nc.gpsimd.indirect_dma_start(
                out=G2[:], out_offset=None, in_=tab,
                in_offset=bass.IndirectOffsetOnAxis(ap=gidx2[:, :1], axis=0))
            for wr in last_writes:
                add_dep_helper(gins.ins, wr.ins, reason="table RAW")
                add_dep_helper(gins2.ins, wr.ins, reason="table RAW")

        if do_book:
            # -- masks + pointer updates (independent of the compose results;
            #    run them here so they overlap the gather/matmul) --
            ohs_d = itk.tile([NP3, 32], FP32, tag="ohsd")
            nc.vector.tensor_scalar(ohs_d[:], iota_sb[:], sstar[:], dcol,
                                    OP.is_equal, OP.mult)
            ohs_d8 = itk.tile([NP3, 32], U8, tag="ohsd8")
            nc.vector.tensor_copy(ohs_d8[:], ohs_d[:])
            m1 = itk.tile([NP3, 32], FP32, tag="m1")
            nc.vector.tensor_scalar(m1[:], iota_sb[:], pstar[:], dcol,
                                    OP.is_equal, OP.mult)
            m1_8 = itk.tile([NP3, 32], U8, tag="m18")
            nc.vector.tensor_copy(m1_8[:], m1[:])
            ohn_d8 = itk.tile([NP3, 32], U8, tag="ohnd8")
            nc.vector.tensor_scalar(tmp[:], iota_sb[:], nstar[:], dcol,
                                    OP.is_equal, OP.mult)
            nc.vector.tensor_copy(ohn_d8[:], tmp[:])
            ohn2_d8 = itk.tile([NP3, 32], U8, tag="ohn2d8")
            nc.vector.tensor_scalar(tmp[:], iota_sb[:], n2[:], dcol,
                                    OP.is_equal, OP.mult)
            nc.vector.tensor_copy(ohn2_d8[:], tmp[:])
            ex2 = itk.tile([NP3, 1], FP32, tag="ex2")
            nc.vector.tensor_scalar(ex2[:], n2[:], SENT, None, OP.is_lt)

            # loc[s*] = ci ; nxt[s*] = n2 ; prv[n2] = s* ; cidx[p*/s*] = e1/e2
            e1v, e2v = float(63 + 2 * i), float(64 + 2 * i)
            nc.vector.copy_predicated(loc_t[:], ohs_d8[:],
                                      ci[:].to_broadcast([NP3, 32]))
            nc.vector.copy_predicated(nxt_t[:], ohs_d8[:],
                                      n2[:].to_broadcast([NP3, 32]))
            nc.vector.copy_predicated(prv_t[:], ohn2_d8[:],
                                      sstar[:].to_broadcast([NP3, 32]))
            cst = itk.tile([NP3, 1], FP32, tag="cst")
            nc.vector.memset(cst[:], e1v)
            nc.vector.copy_predicated(cidx_t[:], m1_8[:],
                                      cst[:].to_broadcast([NP3, 32]))
            cst2 = itk.tile([NP3, 1], FP32, tag="cst2")
            nc.vector.memset(cst2[:], e2v)
            nc.vector.copy_predicated(cidx_t[:], ohs_d8[:],
                                      cst2[:].to_broadcast([NP3, 32]))

        # -- 5/6. packed hi/lo bf16 compose, b-major output --
        # Stationary per k-tile: [128, 128] = [x_hi(64 pairs) | x_lo(64)]
        # in bf16; moving: W^T hi/lo chunks [128, 512] per gate. psum rows
        # 0:64 collect hi*(Whi+Wlo), rows 64:128 collect lo*(Whi+Wlo); their
        # sum is the exact 4-term split product (fp32-quality, 0 argmax
        # flips; single-pass bf16/fp16 DOES flip merges).
        # pair1=(E1,E2) rows G[0:64], pair2=(E2,E3) rows G[32:96].
        if do_compose:
            xsp = itp.tile([128, 8, 128], BF16, tag="xsp")
            for kt in range(8):
                pt = trs.tile([128, 128], FP32, tag="pt")
                src = (G[0:64, kt * 128:(kt + 1) * 128] if kt < 4
                       else G2[0:64, (kt - 4) * 128:(kt - 3) * 128])
                nc.tensor.transpose(pt[:, 0:64], src, ident[0:64, 0:64])
                nc.scalar.copy(xsp[:, kt, 0:64], pt[:, 0:64])
                nc.vector.tensor_tensor(xsp[:, kt, 64:128], pt[:, 0:64],
                                        xsp[:, kt, 0:64], OP.subtract)
        ps_list = []
        for g in range(5):
            if do_compose:
                ps = gps.tile([128, 512], FP32, tag=f"pg{g}")
                nc.tensor.matmul(ps[:], bstat_sb[0:1, :],
                                 bcrow_sb[0:1, g * 512:(g + 1) * 512],
                                 start=True, stop=False)
                ps_list.append(ps)
            else:
                ps_list.append(abl_psd)
        if do_compose:
            for kt in range(8):
                st = xsp[:, kt, :]
                for g in range(5):
                    nc.tensor.matmul(
                        ps_list[g][:], st,
                        wcbh_sb[:, kt, g * 512:(g + 1) * 512],
                        start=False, stop=False)
                    nc.tensor.matmul(
                        ps_list[g][:], st,
                        wcbl_sb[:, kt, g * 512:(g + 1) * 512],
                        start=False, stop=(kt == 7))
        # hccand rows: 0:32 pair1 per b, 32:64 pair2 per b; cols h|c
        hccand = itp.tile([64, 2 * H], FP32, tag="hcc")
        if do_acts:
            acts = []
            for g, ps in enumerate(ps_list):
                # fold the lo-row half back into rows 0:64 on the PE:
                # stage ps[64:128] to SBUF, then accumulate it into the same
                # psum bank via an identity matmul (engines cannot move data
                # across partitions; the PE write port can).
                vlo = itk.tile([128, 512], FP32, tag=f"vl{g}")
                if g % 2 == 0:
                    nc.scalar.copy(vlo[64:128, :], ps[64:128, :])
                else:
                    nc.vector.tensor_copy(vlo[64:128, :], ps[64:128, :])
                if do_compose:
                    nc.tensor.matmul(ps[0:64, :], ident[64:128, 64:128],
                                     vlo[64:128, :], start=False, stop=True,
                                     skip_group_check=True)
                a = itk.tile([64, 512], FP32, tag=f"ga{g}")
                nc.scalar.activation(a[:], ps[0:64, :],
                                     AF.Tanh if g == 3 else AF.Sigmoid)
                acts.append(a)
            si, t1, t2, t3, so = acts
            cl = G[0:64, 512:1024]
            cr = G2[0:64, 512:1024]
            cn = hccand[:, H:2 * H]
            hn = hccand[:, 0:H]
            nc.vector.tensor_tensor(cn, t1[:], cl, OP.mult)
            nc.gpsimd.tensor_tensor(t2[:], t2[:], cr, OP.mult)
            nc.gpsimd.tensor_tensor(cn, cn, t2[:], OP.add)
            nc.gpsimd.tensor_tensor(t3[:], t3[:], si[:], OP.mult)
            nc.vector.tensor_tensor(cn, cn, t3[:], OP.add)
            tcn = itk.tile([64, 512], FP32, tag="tg")
            nc.scalar.activation(tcn[:], cn, AF.Tanh)
            nc.vector.tensor_tensor(hn, so[:], tcn[:], OP.mult)

            # -- 7. fresh logits: per-pair dot with q, then broadcast to the
            # 3 replicated bookkeeping groups via transpose + rank-1 matmul --
            lq = itk.tile([64, 512], FP32, tag="lq")
            nc.gpsimd.tensor_tensor(lq[:], hn, qrow_sb[:], OP.mult)
            l64 = itk.tile([64, 1], FP32, tag="l64")
            nc.vector.tensor_reduce(l64[:], lq[:], AX.X, OP.add)
            ltp = trs.tile([128, 128], FP32, tag="pt")
            nc.tensor.transpose(ltp[0:1, 0:64], l64[:], ident[0:64, 0:64])
            lrow = itk.tile([1, 64], FP32, tag="lrow")
            nc.vector.tensor_copy(lrow[:], ltp[0:1, 0:64])
            plb = ops.tile([NP3, 64], FP32, tag="pl")
            nc.tensor.matmul(plb[:], ones_sb[0:1, 0:NP3], lrow[0:1, :],
                             start=True, stop=True)
            lmt = itk.tile([NP3, 64], FP32, tag="lmt")
            nc.vector.tensor_tensor(lmt[:], plb[:], md2_sb[:], OP.mult)
            lnew = itk.tile([NP3, 2], FP32, tag="lnew")
            nc.vector.tensor_reduce(
                lnew[:], lmt[:].rearrange("p (r b) -> p r b", r=2), AX.X,
                OP.add)
            if dbg is not None:
                nc.sync.dma_start(out=dbg["ln"][i], in_=lnew[:])
        else:
            lnew = abl_lnew
            nc.vector.memset(hccand[:], 0.2)

        if do_dma:
            # -- 8. write fresh candidate rows to table (already b-major) --
            w1 = qdma(out=tabv_eb[63 + 2 * i, 0:ab, :],
                      in_=hccand[0:ab, :])
            w2 = qdma(out=tabv_eb[64 + 2 * i, 0:ab, :],
                      in_=hccand[32:32 + ab, :])
            last_writes = [w1, w2]

        if do_book:
            # -- 9. logit updates (only remaining post-compose bookkeeping) --
            # l[p*] = v1 ; l[s*] = v2 if n2 exists else NEG ; l[n*] = NEG
            nc.vector.copy_predicated(l_t[:], m1_8[:],
                                      lnew[:, 0:1].to_broadcast([NP3, 32]))
            # v2p = v2*ex2 + NEG*(1-ex2), avoiding 1e9 absorption of v2
            v2p = itk.tile([NP3, 1], FP32, tag="v2p")
            nc.vector.tensor_tensor(v2p[:], lnew[:, 1:2], ex2[:], OP.mult)
            negpart = itk.tile([NP3, 1], FP32, tag="negpart")
            nc.vector.tensor_scalar(negpart[:], ex2[:], 1.0, -NEG,
                                    OP.subtract, OP.mult)
            nc.vector.tensor_tensor(v2p[:], v2p[:], negpart[:], OP.add)
            nc.vector.copy_predicated(l_t[:], ohs_d8[:],
                                      v2p[:].to_broadcast([NP3, 32]))
            nc.vector.copy_predicated(l_t[:], ohn_d8[:],
                                      negc_sb[:].to_broadcast([NP3, 32]))

    # ================= output =================
    oidx = itk.tile([BC, 1], FP32, tag="oidx")
    nc.vector.tensor_tensor(oidx[:], loc_t[0:BC, 0:1], bcol_sb[0:BC, :],
                            OP.add)
    oidxu = itk.tile([BC, 1], U32, tag="oidxu")
    nc.vector.tensor_copy(oidxu[:], oidx[:])
    Gout = itp.tile([BC, 2 * H], FP32, tag="Gout")
    gout_ins = nc.gpsimd.indirect_dma_start(
        out=Gout[:], out_offset=None, in_=tab,
        in_offset=bass.IndirectOffsetOnAxis(ap=oidxu[:, :1], axis=0))
    for wr in last_writes:
        add_dep_helper(gout_ins.ins, wr.ins, reason="table RAW")
    nc.sync.dma_start(out=out_d, in_=Gout[:, 0:H])


_BUILD_CACHE = {}


def build(sched=None, max_iters=None, ablate=()):
    if sched is None:
        sched = (32,) * NIC
    sched = tuple(int(v) for v in sched)
    ablate = tuple(ablate)
    key = (sched, max_iters, ablate)
    if key in _BUILD_CACHE:
        return _BUILD_CACHE[key]
    nc = bacc.Bacc("TRN2", target_bir_lowering=False, debug=False)
    io = {
        "xT": nc.dram_tensor("xT", [4, 128, BC * L], FP32, kind="ExternalInput").ap(),
        "wwT": nc.dram_tensor("wwT", [4, 128, 2 * H], FP32, kind="ExternalInput").ap(),
        "wcT": nc.dram_tensor("wcT", [8, 128, 5 * H], FP32, kind="ExternalInput").ap(),
        "wcbh": nc.dram_tensor("wcbh", [8, 128, 5 * H], BF16, kind="ExternalInput").ap(),
        "wcbl": nc.dram_tensor("wcbl", [8, 128, 5 * H], BF16, kind="ExternalInput").ap(),
        "qrow": nc.dram_tensor("qrow", [64, H], FP32, kind="ExternalInput").ap(),
        "bw": nc.dram_tensor("bw", [128, 8], FP32, kind="ExternalInput").ap(),
        "bc": nc.dram_tensor("bc", [128, 20], FP32, kind="ExternalInput").ap(),
        "bcrow": nc.dram_tensor("bcrow", [1, 5 * H], FP32, kind="ExternalInput").ap(),
        "qrep": nc.dram_tensor("qrep", [128, 4, NP3], FP32, kind="ExternalInput").ap(),
        "tab": nc.dram_tensor("tab", [BC * NE, 2 * H], FP32, kind="ExternalInput").ap(),
        "iota32": nc.dram_tensor("iota32", [NP3, 32], FP32, kind="ExternalInput").ap(),
        "bcol": nc.dram_tensor("bcol", [NP3, 1], FP32, kind="ExternalInput").ap(),
        "mdiag2": nc.dram_tensor("mdiag2", [NP3, 64], FP32, kind="ExternalInput").ap(),
        "mdiag31": nc.dram_tensor("mdiag31", [NP3, NIC * BC], FP32, kind="ExternalInput").ap(),
        "lmask": nc.dram_tensor("lmask", [NP3, 32], FP32, kind="ExternalInput").ap(),
        "nxt0": nc.dram_tensor("nxt0", [NP3, 32], FP32, kind="ExternalInput").ap(),
        "prv0": nc.dram_tensor("prv0", [NP3, 32], FP32, kind="ExternalInput").ap(),
        "done": nc.dram_tensor("done", [NP3, NIC], FP32, kind="ExternalInput").ap(),
        "out": nc.dram_tensor("out", [BC, H], FP32, kind="ExternalOutput").ap(),
    }
    if DEBUG:
        io["dbg"] = {
            "l": nc.dram_tensor("dbg_l", [NIC, NP3, 32], FP32, kind="ExternalOutput").ap(),
            "ss": nc.dram_tensor("dbg_ss", [NIC, NP3, 1], FP32, kind="ExternalOutput").ap(),
            "gs": nc.dram_tensor("dbg_gs", [NIC, NP3, 1], FP32, kind="ExternalOutput").ap(),
            "ln": nc.dram_tensor("dbg_ln", [NIC, NP3, 2], FP32, kind="ExternalOutput").ap(),
        }
    with tile.TileContext(nc) as tc:
        with ExitStack() as ctx:
            build_kernel(ctx, tc, io, sched, max_iters=max_iters,
                         ablate=ablate)
    nc.compile()
    _BUILD_CACHE[key] = nc
    return nc


def make_sched(length):
    length = np.asarray(length).astype(np.int64)
    cnt = [(length > i + 1).sum() for i in range(NIC)]
    return tuple(int(-(-c // NCORES)) for c in cnt)


def make_order(length):
    length = np.asarray(length).astype(np.int64)
    order = np.argsort(-length, kind="stable")
    return order.reshape(L, NCORES)


def make_in_maps(x, length, W_word, b_word, W_comp, b_comp, comp_query):
    x = np.asarray(x, np.float32)
    length = np.asarray(length).astype(np.int64)
    W_word = np.asarray(W_word, np.float32)
    b_word = np.asarray(b_word, np.float32)
    W_comp = np.asarray(W_comp, np.float32)
    b_comp = np.asarray(b_comp, np.float32)
    comp_query = np.asarray(comp_query, np.float32)

    import ml_dtypes
    wwT = np.ascontiguousarray(W_word.T.reshape(4, 128, 2 * H))
    wcT = np.ascontiguousarray(W_comp.T.reshape(8, 128, 5 * H))
    wcbh = wcT.astype(ml_dtypes.bfloat16)
    wcbl = (wcT - wcbh.astype(np.float32)).astype(ml_dtypes.bfloat16)
    bw = np.ascontiguousarray(b_word.reshape(8, 128).T)
    bca = b_comp.copy()
    bca[H:3 * H] += 1.0
    bc = np.ascontiguousarray(bca.reshape(20, 128).T)
    qs = (comp_query * (1.0 / np.sqrt(H))).astype(np.float32)
    qrep = np.ascontiguousarray(
        np.broadcast_to(qs.reshape(4, 128, 1), (4, 128, NP3))
        .transpose(1, 0, 2)).astype(np.float32)
    qrow = np.ascontiguousarray(
        np.broadcast_to(qs.reshape(1, H), (64, H))).astype(np.float32)

    iota32 = np.tile(np.arange(32, dtype=np.float32), (NP3, 1))
    bcol = (np.arange(NP3, dtype=np.float32) % BC).reshape(NP3, 1) * NE
    bidx = np.arange(NP3) % BC
    md2 = np.zeros((NP3, 64), np.float32)
    md2[np.arange(NP3), bidx] = 1.0
    md2[np.arange(NP3), 32 + bidx] = 1.0
    md31 = np.zeros((NP3, NIC, BC), np.float32)
    md31[np.arange(NP3), :, bidx] = 1.0
    md31 = md31.reshape(NP3, NIC * BC)
    tabz = np.zeros((BC * NE, 2 * H), np.float32)

    ordmat = make_order(length)
    in_maps = []
    for k in range(NCORES):
        idxs = ordmat[:, k]
        xs = x[idxs]
        xT = np.ascontiguousarray(xs.transpose(2, 0, 1).reshape(4, 128, BC * L))
        ln = length[idxs].astype(np.int64)
        lnr = ln[bidx]  # [96]
        lmask = (np.arange(32)[None, :] < (lnr[:, None] - 1)).astype(np.float32)
        lmask[:, 31] = 0.0
        nxt0 = np.full((NP3, 32), SENT, np.float32)
        prv0 = np.full((NP3, 32), SENT, np.float32)
        for p in range(NP3):
            m = int(lnr[p])
            for s in range(m - 1):
                nxt0[p, s] = s + 1
            for s in range(1, m):
                prv0[p, s] = s - 1
        done = (np.arange(1, L)[None, :] < lnr[:, None]).astype(np.float32)
        in_maps.append({
            "xT": xT, "wwT": wwT, "wcT": wcT, "wcbh": wcbh, "wcbl": wcbl,
            "qrow": qrow, "bw": bw, "bc": bc,
            "bcrow": np.ascontiguousarray(bca.reshape(1, 5 * H)),
            "qrep": qrep, "tab": tabz, "iota32": iota32, "bcol": bcol,
            "mdiag2": md2, "mdiag31": md31, "lmask": lmask,
            "nxt0": nxt0, "prv0": prv0, "done": done,
        })
    return in_maps


def kernel(x, length, W_word, b_word, W_comp, b_comp, comp_query):
    nc = build(make_sched(length))
    in_maps = make_in_maps(x, length, W_word, b_word, W_comp, b_comp, comp_query)
    res = run_bass_kernel_spmd(nc, in_maps, list(range(NCORES)))
    out = np.zeros((B, H), np.float32)
    ordmat = make_order(length)
    for k in range(NCORES):
        out[ordmat[:, k]] = res.results[k]["out"]
    return out



# revision 47
# speedup vs baseline: 11.2703x; 2.7844x over previous
"""Bass/Tile TRN2 kernel for nn_BinaryTreeLSTM (B=256, L=32, D=512, H=512).

Incremental greedy TreeLSTM, data-parallel over batch (32 seqs/core, 8 cores).

Instead of recomputing all adjacent-pair compositions each of the 31 shrink
iterations (O(L^2) matmul work), maintains per-sequence state in "slot space"
(no physical shifting) plus an entity table of h|c value rows in DRAM:
  rows b*128+e: e in [0,32) leaf states, [32,63) initial pair candidates,
  [63,125) fresh candidates (2 per iteration).
Each iteration: argmax over maintained logits -> linked-list bookkeeping row
ops -> two indirect-DMA gathers of entity rows ([E1|E2] and [E2|E3], base-0
aligned so no engine op ever crosses partition lanes) -> compose -> gating ->
fresh logits + masked scatter updates -> table write.

The compose is a packed hi/lo bf16 matmul in transposed orientation:
stationary per k-tile is [x_hi(64 pairs) | x_lo(64)] in bf16, moving is
W_comp^T split host-side into bf16 hi+lo. PSUM rows 0:64 accumulate
hi*(Whi+Wlo), rows 64:128 lo*(Whi+Wlo); the lo half is folded back into rows
0:64 with an identity matmul (PE write port is the only cross-partition
path). The sum is the exact 4-term split product -- fp32-quality logits.
This matters: the greedy argmax has top-2 gaps down to 2e-7, so any
single-pass bf16/fp16/tf32 matmul flips merge decisions (measured: 23/256
sequences flip in bf16 -> rel err 7e-2). The 4-term split keeps logit error
~1e-8 and flips nothing, while cutting the per-iteration compose from ~181
small fp32 matmuls (ld-weights bound, ~93us/iter on HW) to ~90 wide bf16
ones (~7us/iter). Outputs land b-major so table writes need no transposes.
"""

import math
import sys
import numpy as np

sys.path.insert(0, "/opt/trn_rl_repo")

from contextlib import ExitStack

import concourse.bass as bass
import concourse.tile as tile
from concourse import bacc, mybir
from concourse.bass_utils import run_bass_kernel_spmd
from concourse.masks import make_identity
from concourse.tile import add_dep_helper

FP32 = mybir.dt.float32
BF16 = mybir.dt.bfloat16
U32 = mybir.dt.uint32
U8 = mybir.dt.uint8

B, L, D, H = 256, 32, 512, 512
NCORES = 8
BC = B // NCORES          # 32 sequences per core
NE = 128                  # entity rows per sequence in the table
NEG = -1.0e9
SENT = 33.0
NP3 = 64                  # 2 replicated bookkeeping groups of 32 partitions
AF = mybir.ActivationFunctionType
OP = mybir.AluOpType
AX = mybir.AxisListType
NIC = L - 1               # 31 initial candidates / iterations
DEBUG = False


def build_kernel(ctx: ExitStack, tc: "tile.TileContext", io: dict, sched,
                 max_iters=None, ablate=()):
    # ablate: timing-only experiment knobs ("compose", "dma", "book", "acts")
    nc = tc.nc
    imax = max(i for i in range(NIC) if sched[i] >= 1) + 1
    if max_iters is not None:
        imax = min(imax, max_iters)

    # rotate bulk DMAs across engine queues; gathers get explicit deps
    dmaq = [nc.sync, nc.scalar]
    qi = [0]

    def qdma(**kw):
        eng = dmaq[qi[0] % 2]
        qi[0] += 1
        return eng.dma_start(**kw)

    xT = io["xT"]          # [4, 128, BC*L]
    wwT = io["wwT"]        # [4, 128, 2H]
    wcT = io["wcT"]        # [8, 128, 5H]
    bw_d = io["bw"]        # [128, 8]
    bc_d = io["bc"]        # [128, 20]  (+1.0 folded into fl/fr)
    qrep_d = io["qrep"]    # [128, 4, 96] scaled comp_query replicated over M
    tab = io["tab"]        # [BC*NE, 2H] value table (zero-filled)
    iota_d = io["iota32"]  # [96, 32]
    bcol_d = io["bcol"]    # [96, 1]  (p%32)*NE
    md2_d = io["mdiag2"]   # [96, 64]
    md31_d = io["mdiag31"] # [96, 992]
    lmask_d = io["lmask"]  # [96, 32]
    nxt0_d = io["nxt0"]    # [96, 32]
    prv0_d = io["prv0"]    # [96, 32]
    done_d = io["done"]    # [96, 31]
    out_d = io["out"]      # [BC, H]

    tabv_be = tab.rearrange("(b e) d -> b e d", b=BC)   # [32, 128, 1024]
    tabv_eb = tab.rearrange("(b e) d -> e b d", b=BC)   # [128, 32, 1024]

    consts = ctx.enter_context(tc.tile_pool(name="consts", bufs=1))
    state = ctx.enter_context(tc.tile_pool(name="state", bufs=1))

    # ---- persistent constants ----
    bw_sb = consts.tile([128, 8], FP32, tag="bw")
    nc.sync.dma_start(out=bw_sb[:], in_=bw_d[:])
    bc_sb = consts.tile([128, 20], FP32, tag="bc")
    nc.sync.dma_start(out=bc_sb[:], in_=bc_d[:])
    qrep_sb = consts.tile([128, 4, NP3], FP32, tag="qrep")
    nc.sync.dma_start(out=qrep_sb[:], in_=qrep_d[:])
    iota_sb = consts.tile([NP3, 32], FP32, tag="iota")
    nc.sync.dma_start(out=iota_sb[:], in_=iota_d[:])
    bcol_sb = consts.tile([NP3, 1], FP32, tag="bcol")
    nc.sync.dma_start(out=bcol_sb[:], in_=bcol_d[:])
    md2_sb = consts.tile([NP3, 64], FP32, tag="md2")
    nc.sync.dma_start(out=md2_sb[:], in_=md2_d[:])
    lmask_sb = consts.tile([NP3, 32], FP32, tag="lmask")
    nc.sync.dma_start(out=lmask_sb[:], in_=lmask_d[:])
    done_sb = consts.tile([NP3, NIC], FP32, tag="done")
    nc.sync.dma_start(out=done_sb[:], in_=done_d[:])
    negc_sb = consts.tile([NP3, 1], FP32, tag="negc")
    nc.vector.memset(negc_sb[:], NEG)
    bcrow_sb = consts.tile([1, 5 * H], FP32, tag="bcrow")
    nc.sync.dma_start(out=bcrow_sb[:], in_=io["bcrow"])
    ones_sb = consts.tile([1, 96], FP32, tag="ones")
    nc.vector.memset(ones_sb[:], 1.0)
    # bias selector for the packed compose: 1 on hi rows (m<64), 0 on lo rows
    bstat_sb = consts.tile([1, 128], FP32, tag="bstat")
    nc.vector.memset(bstat_sb[:, 0:64], 1.0)
    nc.vector.memset(bstat_sb[:, 64:128], 0.0)
    ident = consts.tile([128, 128], FP32, tag="ident")
    make_identity(nc, ident[:])

    # ---- bookkeeping state (3 replicated groups of 32 partitions) ----
    l_t = state.tile([NP3, 32], FP32, tag="l")
    nxt_t = state.tile([NP3, 32], FP32, tag="nxt")
    nc.sync.dma_start(out=nxt_t[:], in_=nxt0_d[:])
    prv_t = state.tile([NP3, 32], FP32, tag="prv")
    nc.sync.dma_start(out=prv_t[:], in_=prv0_d[:])
    loc_t = state.tile([NP3, 32], FP32, tag="loc")
    nc.vector.tensor_copy(loc_t[:], iota_sb[:])
    cidx_t = state.tile([NP3, 32], FP32, tag="cidx")
    nc.vector.tensor_scalar(cidx_t[:], iota_sb[:], 32.0, None, OP.add)
    nc.vector.memset(cidx_t[:, 31:32], 0.0)

    # ================= init phase =================
    with tc.tile_pool(name="initp", bufs=1) as initp, \
         tc.tile_pool(name="initx", bufs=2) as initx, \
         tc.tile_pool(name="initxb", bufs=1) as initxb, \
         tc.tile_pool(name="initps", bufs=5, space="PSUM") as initps, \
         tc.tile_pool(name="inittr", bufs=1, space="PSUM") as inittr:

        wc_sb = initp.tile([128, 8, 5 * H], FP32, tag="wc")
        for kt in range(8):
            nc.sync.dma_start(out=wc_sb[:, kt, :], in_=wcT[kt])
        ww_sb = initp.tile([128, 4, 2 * H], FP32, tag="ww")
        for kt in range(4):
            nc.sync.dma_start(out=ww_sb[:, kt, :], in_=wwT[kt])
        md31_sb = initp.tile([NP3, 992], FP32, tag="md31")
        nc.sync.dma_start(out=md31_sb[:], in_=md31_d[:])

        # ---- leaves: hc = W_word @ x ; layout [128, f, b, s] ----
        hleaf = initp.tile([128, 4, BC, L], FP32, tag="hleaf")
        cleaf = initp.tile([128, 4, BC, L], FP32, tag="cleaf")
        if "init_leafmm" in ablate:
            nc.vector.memset(hleaf[:], 0.1)
            nc.vector.memset(cleaf[:], 0.1)
        else:
            for cidx2 in range(2):
                xb = []
                for kt in range(4):
                    xt = initxb.tile([128, 512], FP32, tag=f"xb{kt}")
                    nc.sync.dma_start(
                        out=xt[:], in_=xT[kt][:, cidx2 * 512:(cidx2 + 1) * 512])
                    xb.append(xt)
                for m in range(8):
                    ps = initps.tile([128, 512], FP32, tag="pg")
                    for kt in range(4):
                        nc.tensor.matmul(
                            ps[:], ww_sb[:, kt, m * 128:(m + 1) * 128],
                            xb[kt][:], start=(kt == 0), stop=(kt == 3))
                    dst = hleaf if m < 4 else cleaf
                    dview = dst[:, m % 4, 16 * cidx2:16 * cidx2 + 16, :]
                    nc.vector.tensor_scalar(
                        dview, ps[:].rearrange("p (b n) -> p b n", b=16),
                        bw_sb[:, m:m + 1], None, OP.add)

        # ---- write leaf rows to table (b-major) ----
        # per-b 2D DMAs: 3D APs (and degenerate [1,1] dims from rearrange)
        # shatter descriptor merging and cost ~30x on the DMA engines.
        init_writes = []
        if "init_leafwr" not in ablate:
            for blk in range(8):
                bm = initxb.tile([128, 1024], FP32, tag="bm")
                for m in range(8):
                    src = hleaf if m < 4 else cleaf
                    sv = src[:, m % 4].rearrange("p b s -> p (b s)")
                    pt = inittr.tile([128, 128], FP32, tag="pt")
                    nc.tensor.transpose(
                        pt[:], sv[:, blk * 128:(blk + 1) * 128], ident[:])
                    nc.vector.tensor_copy(bm[:, m * 128:(m + 1) * 128], pt[:])
                for j in range(4):
                    init_writes.append(qdma(
                        out=tabv_be[4 * blk + j, 0:32, :],
                        in_=bm[32 * j:32 * j + 32, :]))

        # ---- initial candidates: compose valid adjacent pairs, (s,b) order ----
        candh = initp.tile([128, 4, NIC * BC], FP32, tag="candh")
        candc = initp.tile([128, 4, NIC * BC], FP32, tag="candc")
        nc.vector.memset(candh[:], 0.0)
        nc.vector.memset(candc[:], 0.0)

        def leaf_v(t, kt, s0, s1, bh):
            return t[:, kt].rearrange("p b s -> p s b")[:, s0:s1, 0:bh]

        NTOT = NIC * BC  # 992
        # s-chunks with per-chunk active-b bound from the baked schedule
        chunks = []
        s0 = 0
        while s0 < imax:
            bh = max(sched[s0], 1)
            s1 = s0 + 1
            while s1 < imax and (s1 + 1 - s0) * bh <= 512:
                s1 += 1
            chunks.append((s0, s1, bh))
            s0 = s1
        for (sc0, sc1, bh) in (() if "init_cand" in ablate else chunks):
            ncol = (sc1 - sc0) * bh
            for f in range(4):
                ps_g = []
                for g in range(5):
                    mt = g * 4 + f
                    ps = initps.tile([128, 512], FP32, tag="pg")
                    for kt in range(8):
                        if kt < 4:
                            rv = leaf_v(hleaf, kt, sc0, sc1, bh)
                        else:
                            rv = leaf_v(hleaf, kt - 4, sc0 + 1, sc1 + 1, bh)
                        nc.tensor.matmul(
                            ps[:, 0:ncol],
                            wc_sb[:, kt, mt * 128:(mt + 1) * 128], rv,
                            start=(kt == 0), stop=(kt == 7))
                    ps_g.append(ps)
                cl = leaf_v(cleaf, f, sc0, sc1, bh)
                cr = leaf_v(cleaf, f, sc0 + 1, sc1 + 1, bh)
                cn = candc[:, f].rearrange("p (s b) -> p s b", b=BC)[:, sc0:sc1, 0:bh]
                hn = candh[:, f].rearrange("p (s b) -> p s b", b=BC)[:, sc0:sc1, 0:bh]
                def v3(t):
                    return t[:, 0:ncol].rearrange("p (s b) -> p s b", b=bh)

                si = initx.tile([128, 512], FP32, tag="si")
                nc.scalar.activation(si[:, 0:ncol], ps_g[0][:, 0:ncol],
                                     AF.Sigmoid, bias=bc_sb[:, f:f + 1])
                t1 = initx.tile([128, 512], FP32, tag="tg")
                nc.scalar.activation(t1[:, 0:ncol], ps_g[1][:, 0:ncol],
                                     AF.Sigmoid, bias=bc_sb[:, 4 + f:5 + f])
                nc.vector.tensor_tensor(cn, v3(t1), cl, OP.mult)
                t2 = initx.tile([128, 512], FP32, tag="tg")
                nc.scalar.activation(t2[:, 0:ncol], ps_g[2][:, 0:ncol],
                                     AF.Sigmoid, bias=bc_sb[:, 8 + f:9 + f])
                nc.gpsimd.tensor_tensor(v3(t2), v3(t2), cr, OP.mult)
                nc.gpsimd.tensor_tensor(cn, cn, v3(t2), OP.add)
                t3 = initx.tile([128, 512], FP32, tag="tg")
                nc.scalar.activation(t3[:, 0:ncol], ps_g[3][:, 0:ncol],
                                     AF.Tanh, bias=bc_sb[:, 12 + f:13 + f])
                nc.gpsimd.tensor_tensor(v3(t3), v3(t3), v3(si), OP.mult)
                nc.gpsimd.tensor_tensor(cn, cn, v3(t3), OP.add)
                so = initx.tile([128, 512], FP32, tag="so")
                nc.scalar.activation(so[:, 0:ncol], ps_g[4][:, 0:ncol],
                                     AF.Sigmoid, bias=bc_sb[:, 16 + f:17 + f])
                tcn = initx.tile([128, 512], FP32, tag="tg")
                nc.scalar.activation(v3(tcn), cn, AF.Tanh)
                nc.vector.tensor_tensor(hn, v3(so), v3(tcn), OP.mult)

        # ---- init logits ----
        nc.vector.memset(l_t[:], NEG)
        md31v = md31_sb[:].rearrange("p (s b) -> p s b", b=BC)
        for (sc0, sc1, bh) in (() if "init_logit" in ablate else chunks):
            ncol = (sc1 - sc0) * bh
            pl = inittr.tile([NP3, 512], FP32, tag="pl")
            chv = candh[:].rearrange("p f (s b) -> p f s b", b=BC)
            for f in range(4):
                nc.tensor.matmul(
                    pl[:, 0:ncol], qrep_sb[:, f, :],
                    chv[:, f, sc0:sc1, 0:bh],
                    start=(f == 0), stop=(f == 3))
            lm = initxb.tile([NP3, 512], FP32, tag="lm")
            nc.vector.tensor_tensor(
                lm[:, 0:ncol].rearrange("p (s b) -> p s b", b=bh),
                pl[:, 0:ncol].rearrange("p (s b) -> p s b", b=bh),
                md31v[:, sc0:sc1, 0:bh], OP.mult)
            nc.vector.tensor_reduce(
                l_t[:, sc0:sc1],
                lm[:, 0:ncol].rearrange("p (s b) -> p s b", b=bh),
                AX.X, OP.add)
        # mask invalid: l = l*lmask + (1-lmask)*NEG; (lmask-1)*(-NEG) is that term
        tmpl = initx.tile([NP3, 32], FP32, tag="tmpl")
        nc.vector.tensor_scalar(tmpl[:], lmask_sb[:], 1.0, -NEG, OP.subtract,
                                OP.mult)
        nc.vector.tensor_tensor(l_t[:], l_t[:], lmask_sb[:], OP.mult)
        nc.vector.tensor_tensor(l_t[:], l_t[:], tmpl[:], OP.add)

        # ---- write init candidate rows to table ((s,b) order) ----
        for blk in (() if "init_candwr" in ablate else range(8)):
            c0 = blk * 128
            c1 = min(c0 + 128, NTOT)
            w = c1 - c0
            bm = initxb.tile([128, 1024], FP32, tag="bm")
            for m in range(8):
                src = candh if m < 4 else candc
                pt = inittr.tile([128, 128], FP32, tag="pt")
                nc.tensor.transpose(
                    pt[0:w, :], src[:, m % 4, c0:c1], ident[:])
                nc.vector.tensor_copy(bm[0:w, m * 128:(m + 1) * 128], pt[0:w, :])
            s0 = 32 + 4 * blk
            s1 = 32 + min(4 * blk + 4, NIC)
            for j in range(s1 - s0):
                init_writes.append(qdma(
                    out=tabv_eb[s0 + j, 0:32, :],
                    in_=bm[32 * j:32 * j + 32, :]))

    # ================= iterations =================
    itw = ctx.enter_context(tc.tile_pool(name="itw", bufs=1))
    itp = ctx.enter_context(tc.tile_pool(name="itp", bufs=2))
    itk = ctx.enter_context(tc.tile_pool(name="itk", bufs=2))
    gps = ctx.enter_context(tc.tile_pool(name="gps", bufs=1, space="PSUM"))
    trs = ctx.enter_context(tc.tile_pool(name="trs", bufs=2, space="PSUM"))
    ops = ctx.enter_context(tc.tile_pool(name="ops", bufs=1, space="PSUM"))

    # bf16 hi/lo split of W_comp^T for the packed compose (loaded after the
    # init pools release their SBUF)
    wcbh_sb = itw.tile([128, 8, 5 * H], BF16, tag="wcbh")
    wcbl_sb = itw.tile([128, 8, 5 * H], BF16, tag="wcbl")
    for kt in range(8):
        qdma(out=wcbh_sb[:, kt, :], in_=io["wcbh"][kt])
        qdma(out=wcbl_sb[:, kt, :], in_=io["wcbl"][kt])
    qrow_sb = itw.tile([64, H], FP32, tag="qrow")
    nc.sync.dma_start(out=qrow_sb[:], in_=io["qrow"])

    G = itw.tile([64, 2 * H], FP32, tag="G")      # rows [E1(b) | E2(b)]
    G2 = itw.tile([64, 2 * H], FP32, tag="G2")    # rows [E2(b) | E3(b)]
    # --- ablation support (timing-only experiment builds) ---
    if "dma" in ablate:
        nc.vector.memset(G[:], 0.3)
        nc.vector.memset(G2[:], 0.3)
    if "book" in ablate:
        abl_gidx = itw.tile([64, 1], U32, tag="abl_gidx")
        nc.vector.tensor_copy(abl_gidx[:], bcol_sb[0:64, :])
    if "compose" in ablate:
        abl_ps = ctx.enter_context(
            tc.tile_pool(name="ablps", bufs=1, space="PSUM"))
        abl_psd = abl_ps.tile([128, 512], FP32, tag="abl_psd")
        nc.vector.memset(abl_psd[:], 0.1)
    if "acts" in ablate:
        abl_lnew = itw.tile([NP3, 2], FP32, tag="abl_lnew")
        nc.vector.memset(abl_lnew[:], 0.0)

    dbg = io.get("dbg")
    last_writes = init_writes[-3:]
    for i in range(imax):
        ab = min(int(sched[i]), 32)   # active sequences this iteration
        dcol = done_sb[:, i:i + 1]
        if dbg is not None:
            nc.sync.dma_start(out=dbg["l"][i], in_=l_t[:])

        do_book = "book" not in ablate
        do_dma = "dma" not in ablate
        do_compose = "compose" not in ablate
        do_acts = "acts" not in ablate

        if do_book:
            # -- 1. argmax over logits --
            mx8 = itk.tile([NP3, 8], FP32, tag="mx8")
            nc.vector.max(mx8[:], l_t[:])
            ix8 = itk.tile([NP3, 8], U32, tag="ix8")
            nc.vector.max_index(ix8[:], mx8[:], l_t[:])
            sstar = itk.tile([NP3, 1], FP32, tag="sstar")
            nc.vector.tensor_copy(sstar[:], ix8[:, 0:1])

            # -- 2. linked-list row gathers --
            ohs = itk.tile([NP3, 32], FP32, tag="ohs")
            nc.vector.tensor_scalar(ohs[:], iota_sb[:], sstar[:], None,
                                    OP.is_equal)
            tmp = itk.tile([NP3, 32], FP32, tag="tmp")

            def rowgather(arr, oh, name):
                col = itk.tile([NP3, 1], FP32, tag=name)
                nc.vector.tensor_tensor(tmp[:], oh, arr, OP.mult)
                nc.vector.tensor_reduce(col[:], tmp[:], AX.X, OP.max)
                return col

            nstar = rowgather(nxt_t[:], ohs[:], "nstar")
            pstar = rowgather(prv_t[:], ohs[:], "pstar")
            ci = rowgather(cidx_t[:], ohs[:], "ci")
            ohn = itk.tile([NP3, 32], FP32, tag="ohn")
            nc.vector.tensor_scalar(ohn[:], iota_sb[:], nstar[:], None,
                                    OP.is_equal)
            n2 = rowgather(nxt_t[:], ohn[:], "n2")
            ohp = itk.tile([NP3, 32], FP32, tag="ohp")
            nc.vector.tensor_scalar(ohp[:], iota_sb[:], pstar[:], None,
                                    OP.is_equal)
            e1c = rowgather(loc_t[:], ohp[:], "e1c")
            ohn2 = itk.tile([NP3, 32], FP32, tag="ohn2")
            nc.vector.tensor_scalar(ohn2[:], iota_sb[:], n2[:], None,
                                    OP.is_equal)
            e3c = rowgather(loc_t[:], ohn2[:], "e3c")

            # -- 3. gather index columns [64,1]: (E1|E2) and (E2|E3) + b*NE --
            gsel = itk.tile([64, 1], FP32, tag="gsel")
            nc.vector.tensor_copy(gsel[0:32, :], e1c[0:32, :])
            nc.vector.tensor_copy(gsel[32:64, :], ci[32:64, :])
            nc.vector.tensor_tensor(gsel[:], gsel[:], bcol_sb[0:64, :],
                                    OP.add)
            gidx = itk.tile([64, 1], U32, tag="gidx")
            nc.vector.tensor_copy(gidx[:], gsel[:])
            gsel2 = itk.tile([64, 1], FP32, tag="gsel2")
            nc.vector.tensor_copy(gsel2[0:32, :], ci[0:32, :])
            nc.vector.tensor_copy(gsel2[32:64, :], e3c[32:64, :])
            nc.vector.tensor_tensor(gsel2[:], gsel2[:], bcol_sb[0:64, :],
                                    OP.add)
            gidx2 = itk.tile([64, 1], U32, tag="gidx2")
            nc.vector.tensor_copy(gidx2[:], gsel2[:])
            if dbg is not None:
                nc.sync.dma_start(out=dbg["ss"][i], in_=sstar[:])
                nc.sync.dma_start(out=dbg["gs"][i], in_=gsel[:])
        else:
            gidx = abl_gidx
            gidx2 = abl_gidx

        # -- 4. indirect gather of entity rows (must order after table writes;
        #       Tile does not track RAW hazards through DRAM) --
        if do_dma:
            gins = nc.gpsimd.indirect_dma_start(
                out=G[:], out_offset=None, in_=tab,
                in_offset=bass.IndirectOffsetOnAxis(ap=gidx[:, :1], axis=0))
            gins2 = nc.gpsimd.indirect_dma_start(
                out=G2[:], out_offset=None, in_=tab,
                in_offset=bass.IndirectOffsetOnAxis(ap=gidx2[:, :1], axis=0))
            for wr in last_writes:
                add_dep_helper(gins.ins, wr.ins, reason="table RAW")
                add_dep_helper(gins2.ins, wr.ins, reason="table RAW")

        if do_book:
            # -- masks + pointer updates (independent of the compose results;
            #    run them here so they overlap the gather/matmul) --
            ohs_d = itk.tile([NP3, 32], FP32, tag="ohsd")
            nc.vector.tensor_scalar(ohs_d[:], iota_sb[:], sstar[:], dcol,
                                    OP.is_equal, OP.mult)
            ohs_d8 = itk.tile([NP3, 32], U8, tag="ohsd8")
            nc.vector.tensor_copy(ohs_d8[:], ohs_d[:])
            m1 = itk.tile([NP3, 32], FP32, tag="m1")
            nc.vector.tensor_scalar(m1[:], iota_sb[:], pstar[:], dcol,
                                    OP.is_equal, OP.mult)
            m1_8 = itk.tile([NP3, 32], U8, tag="m18")
            nc.vector.tensor_copy(m1_8[:], m1[:])
            ohn_d8 = itk.tile([NP3, 32], U8, tag="ohnd8")
            nc.vector.tensor_scalar(tmp[:], iota_sb[:], nstar[:], dcol,
                                    OP.is_equal, OP.mult)
            nc.vector.tensor_copy(ohn_d8[:], tmp[:])
            ohn2_d8 = itk.tile([NP3, 32], U8, tag="ohn2d8")
            nc.vector.tensor_scalar(tmp[:], iota_sb[:], n2[:], dcol,
                                    OP.is_equal, OP.mult)
            nc.vector.tensor_copy(ohn2_d8[:], tmp[:])
            ex2 = itk.tile([NP3, 1], FP32, tag="ex2")
            nc.vector.tensor_scalar(ex2[:], n2[:], SENT, None, OP.is_lt)

            # loc[s*] = ci ; nxt[s*] = n2 ; prv[n2] = s* ; cidx[p*/s*] = e1/e2
            e1v, e2v = float(63 + 2 * i), float(64 + 2 * i)
            nc.vector.copy_predicated(loc_t[:], ohs_d8[:],
                                      ci[:].to_broadcast([NP3, 32]))
            nc.vector.copy_predicated(nxt_t[:], ohs_d8[:],
                                      n2[:].to_broadcast([NP3, 32]))
            nc.vector.copy_predicated(prv_t[:], ohn2_d8[:],
                                      sstar[:].to_broadcast([NP3, 32]))
            cst = itk.tile([NP3, 1], FP32, tag="cst")
            nc.vector.memset(cst[:], e1v)
            nc.vector.copy_predicated(cidx_t[:], m1_8[:],
                                      cst[:].to_broadcast([NP3, 32]))
            cst2 = itk.tile([NP3, 1], FP32, tag="cst2")
            nc.vector.memset(cst2[:], e2v)
            nc.vector.copy_predicated(cidx_t[:], ohs_d8[:],
                                      cst2[:].to_broadcast([NP3, 32]))

        # -- 5/6. packed hi/lo bf16 compose, b-major output --
        # Stationary per k-tile: [128, 128] = [x_hi(64 pairs) | x_lo(64)]
        # in bf16; moving: W^T hi/lo chunks [128, 512] per gate. psum rows
        # 0:64 collect hi*(Whi+Wlo), rows 64:128 collect lo*(Whi+Wlo); their
        # sum is the exact 4-term split product (fp32-quality, 0 argmax
        # flips; single-pass bf16/fp16 DOES flip merges).
        # pair1=(E1,E2) rows G[0:64], pair2=(E2,E3) rows G[32:96].
        if do_compose:
            xsp = itp.tile([128, 8, 128], BF16, tag="xsp")
            for kt in range(8):
                pt = trs.tile([128, 128], FP32, tag="pt")
                src = (G[0:64, kt * 128:(kt + 1) * 128] if kt < 4
                       else G2[0:64, (kt - 4) * 128:(kt - 3) * 128])
                nc.tensor.transpose(pt[:, 0:64], src, ident[0:64, 0:64])
                nc.scalar.copy(xsp[:, kt, 0:64], pt[:, 0:64])
                nc.vector.tensor_tensor(xsp[:, kt, 64:128], pt[:, 0:64],
                                        xsp[:, kt, 0:64], OP.subtract)
        ps_list = []
        for g in range(5):
            if do_compose:
                ps = gps.tile([128, 512], FP32, tag=f"pg{g}")
                nc.tensor.matmul(ps[:], bstat_sb[0:1, :],
                                 bcrow_sb[0:1, g * 512:(g + 1) * 512],
                                 start=True, stop=False)
                ps_list.append(ps)
            else:
                ps_list.append(abl_psd)
        if do_compose:
            for kt in range(8):
                st = xsp[:, kt, :]
                for g in range(5):
                    nc.tensor.matmul(
                        ps_list[g][:], st,
                        wcbh_sb[:, kt, g * 512:(g + 1) * 512],
                        start=False, stop=False)
                    nc.tensor.matmul(
                        ps_list[g][:], st,
                        wcbl_sb[:, kt, g * 512:(g + 1) * 512],
                        start=False, stop=(kt == 7))
        # hccand rows: 0:32 pair1 per b, 32:64 pair2 per b; cols h|c
        hccand = itp.tile([64, 2 * H], FP32, tag="hcc")
        if do_acts:
            acts = []
            for g, ps in enumerate(ps_list):
                # fold the lo-row half back into rows 0:64 on the PE:
                # stage ps[64:128] to SBUF, then accumulate it into the same
                # psum bank via an identity matmul (engines cannot move data
                # across partitions; the PE write port can).
                vlo = itk.tile([128, 512], FP32, tag=f"vl{g}")
                if g % 2 == 0:
                    nc.scalar.copy(vlo[64:128, :], ps[64:128, :])
                else:
                    nc.vector.tensor_copy(vlo[64:128, :], ps[64:128, :])
                if do_compose:
                    nc.tensor.matmul(ps[0:64, :], ident[64:128, 64:128],
                                     vlo[64:128, :], start=False, stop=True,
                                     skip_group_check=True)
                a = itk.tile([64, 512], FP32, tag=f"ga{g}")
                nc.scalar.activation(a[:], ps[0:64, :],
                                     AF.Tanh if g == 3 else AF.Sigmoid)
                acts.append(a)
            si, t1, t2, t3, so = acts
            cl = G[0:64, 512:1024]
            cr = G2[0:64, 512:1024]
            cn = hccand[:, H:2 * H]
            hn = hccand[:, 0:H]
            nc.vector.tensor_tensor(cn, t1[:], cl, OP.mult)
            nc.gpsimd.tensor_tensor(t2[:], t2[:], cr, OP.mult)
            nc.gpsimd.tensor_tensor(cn, cn, t2[:], OP.add)
            nc.gpsimd.tensor_tensor(t3[:], t3[:], si[:], OP.mult)
            nc.vector.tensor_tensor(cn, cn, t3[:], OP.add)
            tcn = itk.tile([64, 512], FP32, tag="tg")
            nc.scalar.activation(tcn[:], cn, AF.Tanh)
            nc.vector.tensor_tensor(hn, so[:], tcn[:], OP.mult)

            # -- 7. fresh logits: per-pair dot with q, then broadcast to the
            # 3 replicated bookkeeping groups via transpose + rank-1 matmul --
            lq = itk.tile([64, 512], FP32, tag="lq")
            nc.gpsimd.tensor_tensor(lq[:], hn, qrow_sb[:], OP.mult)
            l64 = itk.tile([64, 1], FP32, tag="l64")
            nc.vector.tensor_reduce(l64[:], lq[:], AX.X, OP.add)
            ltp = trs.tile([128, 128], FP32, tag="pt")
            nc.tensor.transpose(ltp[0:1, 0:64], l64[:], ident[0:64, 0:64])
            lrow = itk.tile([1, 64], FP32, tag="lrow")
            nc.vector.tensor_copy(lrow[:], ltp[0:1, 0:64])
            plb = ops.tile([NP3, 64], FP32, tag="pl")
            nc.tensor.matmul(plb[:], ones_sb[0:1, 0:NP3], lrow[0:1, :],
                             start=True, stop=True)
            lmt = itk.tile([NP3, 64], FP32, tag="lmt")
            nc.vector.tensor_tensor(lmt[:], plb[:], md2_sb[:], OP.mult)
            lnew = itk.tile([NP3, 2], FP32, tag="lnew")
            nc.vector.tensor_reduce(
                lnew[:], lmt[:].rearrange("p (r b) -> p r b", r=2), AX.X,
                OP.add)
            if dbg is not None:
                nc.sync.dma_start(out=dbg["ln"][i], in_=lnew[:])
        else:
            lnew = abl_lnew
            nc.vector.memset(hccand[:], 0.2)

        if do_dma:
            # -- 8. write fresh candidate rows to table (already b-major) --
            w1 = qdma(out=tabv_eb[63 + 2 * i, 0:ab, :],
                      in_=hccand[0:ab, :])
            w2 = qdma(out=tabv_eb[64 + 2 * i, 0:ab, :],
                      in_=hccand[32:32 + ab, :])
            last_writes = [w1, w2]

        if do_book:
            # -- 9. logit updates (only remaining post-compose bookkeeping) --
            # l[p*] = v1 ; l[s*] = v2 if n2 exists else NEG ; l[n*] = NEG
            nc.vector.copy_predicated(l_t[:], m1_8[:],
                                      lnew[:, 0:1].to_broadcast([NP3, 32]))
            # v2p = v2*ex2 + NEG*(1-ex2), avoiding 1e9 absorption of v2
            v2p = itk.tile([NP3, 1], FP32, tag="v2p")
            nc.vector.tensor_tensor(v2p[:], lnew[:, 1:2], ex2[:], OP.mult)
            negpart = itk.tile([NP3, 1], FP32, tag="negpart")
            nc.vector.tensor_scalar(negpart[:], ex2[:], 1.0, -NEG,
                                    OP.subtract, OP.mult)
            nc.vector.tensor_tensor(v2p[:], v2p[:], negpart[:], OP.add)
            nc.vector.copy_predicated(l_t[:], ohs_d8[:],
                                      v2p[:].to_broadcast([NP3, 32]))
            nc.vector.copy_predicated(l_t[:], ohn_d8[:],
                                      negc_sb[:].to_broadcast([NP3, 32]))

    # ================= output =================
    oidx = itk.tile([BC, 1], FP32, tag="oidx")
    nc.vector.tensor_tensor(oidx[:], loc_t[0:BC, 0:1], bcol_sb[0:BC, :],
                            OP.add)
    oidxu = itk.tile([BC, 1], U32, tag="oidxu")
    nc.vector.tensor_copy(oidxu[:], oidx[:])
    Gout = itp.tile([BC, 2 * H], FP32, tag="Gout")
    gout_ins = nc.gpsimd.indirect_dma_start(
        out=Gout[:], out_offset=None, in_=tab,
        in_offset=bass.IndirectOffsetOnAxis(ap=oidxu[:, :1], axis=0))
    for wr in last_writes:
        add_dep_helper(gout_ins.ins, wr.ins, reason="table RAW")
    nc.sync.dma_start(out=out_d, in_=Gout[:, 0:H])


_BUILD_CACHE = {}


def build(sched=None, max_iters=None, ablate=()):
    if sched is None:
        sched = (32,) * NIC
    sched = tuple(int(v) for v in sched)
    ablate = tuple(ablate)
    key = (sched, max_iters, ablate)
    if key in _BUILD_CACHE:
        return _BUILD_CACHE[key]
    nc = bacc.Bacc("TRN2", target_bir_lowering=False, debug=False)
    io = {
        "xT": nc.dram_tensor("xT", [4, 128, BC * L], FP32, kind="ExternalInput").ap(),
        "wwT": nc.dram_tensor("wwT", [4, 128, 2 * H], FP32, kind="ExternalInput").ap(),
        "wcT": nc.dram_tensor("wcT", [8, 128, 5 * H], FP32, kind="ExternalInput").ap(),
        "wcbh": nc.dram_tensor("wcbh", [8, 128, 5 * H], BF16, kind="ExternalInput").ap(),
        "wcbl": nc.dram_tensor("wcbl", [8, 128, 5 * H], BF16, kind="ExternalInput").ap(),
        "qrow": nc.dram_tensor("qrow", [64, H], FP32, kind="ExternalInput").ap(),
        "bw": nc.dram_tensor("bw", [128, 8], FP32, kind="ExternalInput").ap(),
        "bc": nc.dram_tensor("bc", [128, 20], FP32, kind="ExternalInput").ap(),
        "bcrow": nc.dram_tensor("bcrow", [1, 5 * H], FP32, kind="ExternalInput").ap(),
        "qrep": nc.dram_tensor("qrep", [128, 4, NP3], FP32, kind="ExternalInput").ap(),
        "tab": nc.dram_tensor("tab", [BC * NE, 2 * H], FP32, kind="ExternalInput").ap(),
        "iota32": nc.dram_tensor("iota32", [NP3, 32], FP32, kind="ExternalInput").ap(),
        "bcol": nc.dram_tensor("bcol", [NP3, 1], FP32, kind="ExternalInput").ap(),
        "mdiag2": nc.dram_tensor("mdiag2", [NP3, 64], FP32, kind="ExternalInput").ap(),
        "mdiag31": nc.dram_tensor("mdiag31", [NP3, NIC * BC], FP32, kind="ExternalInput").ap(),
        "lmask": nc.dram_tensor("lmask", [NP3, 32], FP32, kind="ExternalInput").ap(),
        "nxt0": nc.dram_tensor("nxt0", [NP3, 32], FP32, kind="ExternalInput").ap(),
        "prv0": nc.dram_tensor("prv0", [NP3, 32], FP32, kind="ExternalInput").ap(),
        "done": nc.dram_tensor("done", [NP3, NIC], FP32, kind="ExternalInput").ap(),
        "out": nc.dram_tensor("out", [BC, H], FP32, kind="ExternalOutput").ap(),
    }
    if DEBUG:
        io["dbg"] = {
            "l": nc.dram_tensor("dbg_l", [NIC, NP3, 32], FP32, kind="ExternalOutput").ap(),
            "ss": nc.dram_tensor("dbg_ss", [NIC, NP3, 1], FP32, kind="ExternalOutput").ap(),
            "gs": nc.dram_tensor("dbg_gs", [NIC, NP3, 1], FP32, kind="ExternalOutput").ap(),
            "ln": nc.dram_tensor("dbg_ln", [NIC, NP3, 2], FP32, kind="ExternalOutput").ap(),
        }
    with tile.TileContext(nc) as tc:
        with ExitStack() as ctx:
            build_kernel(ctx, tc, io, sched, max_iters=max_iters,
                         ablate=ablate)
    nc.compile()
    _BUILD_CACHE[key] = nc
    return nc


def make_sched(length):
    length = np.asarray(length).astype(np.int64)
    cnt = [(length > i + 1).sum() for i in range(NIC)]
    return tuple(int(-(-c // NCORES)) for c in cnt)


def make_order(length):
    length = np.asarray(length).astype(np.int64)
    order = np.argsort(-length, kind="stable")
    return order.reshape(L, NCORES)


def make_in_maps(x, length, W_word, b_word, W_comp, b_comp, comp_query):
    x = np.asarray(x, np.float32)
    length = np.asarray(length).astype(np.int64)
    W_word = np.asarray(W_word, np.float32)
    b_word = np.asarray(b_word, np.float32)
    W_comp = np.asarray(W_comp, np.float32)
    b_comp = np.asarray(b_comp, np.float32)
    comp_query = np.asarray(comp_query, np.float32)

    import ml_dtypes
    wwT = np.ascontiguousarray(W_word.T.reshape(4, 128, 2 * H))
    wcT = np.ascontiguousarray(W_comp.T.reshape(8, 128, 5 * H))
    wcbh = wcT.astype(ml_dtypes.bfloat16)
    wcbl = (wcT - wcbh.astype(np.float32)).astype(ml_dtypes.bfloat16)
    bw = np.ascontiguousarray(b_word.reshape(8, 128).T)
    bca = b_comp.copy()
    bca[H:3 * H] += 1.0
    bc = np.ascontiguousarray(bca.reshape(20, 128).T)
    qs = (comp_query * (1.0 / np.sqrt(H))).astype(np.float32)
    qrep = np.ascontiguousarray(
        np.broadcast_to(qs.reshape(4, 128, 1), (4, 128, NP3))
        .transpose(1, 0, 2)).astype(np.float32)
    qrow = np.ascontiguousarray(
        np.broadcast_to(qs.reshape(1, H), (64, H))).astype(np.float32)

    iota32 = np.tile(np.arange(32, dtype=np.float32), (NP3, 1))
    bcol = (np.arange(NP3, dtype=np.float32) % BC).reshape(NP3, 1) * NE
    bidx = np.arange(NP3) % BC
    md2 = np.zeros((NP3, 64), np.float32)
    md2[np.arange(NP3), bidx] = 1.0
    md2[np.arange(NP3), 32 + bidx] = 1.0
    md31 = np.zeros((NP3, NIC, BC), np.float32)
    md31[np.arange(NP3), :, bidx] = 1.0
    md31 = md31.reshape(NP3, NIC * BC)
    tabz = np.zeros((BC * NE, 2 * H), np.float32)

    ordmat = make_order(length)
    in_maps = []
    for k in range(NCORES):
        idxs = ordmat[:, k]
        xs = x[idxs]
        xT = np.ascontiguousarray(xs.transpose(2, 0, 1).reshape(4, 128, BC * L))
        ln = length[idxs].astype(np.int64)
        lnr = ln[bidx]  # [96]
        lmask = (np.arange(32)[None, :] < (lnr[:, None] - 1)).astype(np.float32)
        lmask[:, 31] = 0.0
        nxt0 = np.full((NP3, 32), SENT, np.float32)
        prv0 = np.full((NP3, 32), SENT, np.float32)
        for p in range(NP3):
            m = int(lnr[p])
            for s in range(m - 1):
                nxt0[p, s] = s + 1
            for s in range(1, m):
                prv0[p, s] = s - 1
        done = (np.arange(1, L)[None, :] < lnr[:, None]).astype(np.float32)
        in_maps.append({
            "xT": xT, "wwT": wwT, "wcT": wcT, "wcbh": wcbh, "wcbl": wcbl,
            "qrow": qrow, "bw": bw, "bc": bc,
            "bcrow": np.ascontiguousarray(bca.reshape(1, 5 * H)),
            "qrep": qrep, "tab": tabz, "iota32": iota32, "bcol": bcol,
            "mdiag2": md2, "mdiag31": md31, "lmask": lmask,
            "nxt0": nxt0, "prv0": prv0, "done": done,
        })
    return in_maps


def kernel(x, length, W_word, b_word, W_comp, b_comp, comp_query):
    nc = build(make_sched(length))
    in_maps = make_in_maps(x, length, W_word, b_word, W_comp, b_comp, comp_query)
    res = run_bass_kernel_spmd(nc, in_maps, list(range(NCORES)))
    out = np.zeros((B, H), np.float32)
    ordmat = make_order(length)
    for k in range(NCORES):
        out[ordmat[:, k]] = res.results[k]["out"]
    return out

